# revision 1
# baseline (speedup 1.0000x reference)
"""Self-contained Trainium2 Bass kernel for nn_GCNMagnetModel (3-layer GCN,
N=50000 nodes, E=600000 edges, H=128, 64 graphs, 8 NeuronCores, SPMD 1 NEFF).

Sharding (hardcoded strategy): nodes/edges sharded across 8 cores by graph id
(graphs 8k..8k+7 -> core k; graph-block-aligned node layout so both pools are
core-local). Message passing per dst-block of 128 nodes via one-hot matmuls on
the PE over dma_gather'd rows of a per-layer bf16 table of
hW' = (h @ W) * rsqrt(deg) in partition-major storage (row = k*NMAXP +
(loc%128)*NBLK + loc//128, so shard/table writes are single large
contiguous-per-partition DMAs). GCN norm is separable: dinv_src folded into
the table, dinv_dst applied per dst partition. The layer-1 table is built
locally on every core from replicated x plus one small dinv AllGather (x has
only 2 features), avoiding one of the three large table AllGathers. Degree
counting, rsqrt, all matmuls, tanh, and max/mean pools run on device; the
host only computes index layout (sorting, padding, local renumbering, masks).

kernel(**inputs) -> [64, 41] float32.
"""
import numpy as np
import ml_dtypes
from contextlib import ExitStack

import concourse.tile as tile
import concourse.mybir as mybir
from concourse import bacc
from concourse import library_config
from concourse.bass_utils import run_bass_kernel_spmd

"""(inlined hostprep) Host-side sharding/layout prep for the GCN kernel, plus a numpy emulator
of the exact device dataflow (bf16 table, one-hot matmuls, per-block pipeline)
to validate index bookkeeping and predict accuracy at full scale.

Host only does index/layout manipulation here (sorting, partitioning,
padding, local renumbering, masks, replication of given constants). All FP
math on tensor values happens on-device (emulated in emu_device()).
"""
import numpy as np
import ml_dtypes

NCORE = 8
P = 128
GPC = 8  # graphs per core


def prep(x, edge_index, batch, n_graphs=64):
    N = x.shape[0]
    batch = np.asarray(batch)
    src_g, dst_g = np.asarray(edge_index[0]), np.asarray(edge_index[1])
    E = src_g.shape[0]

    # graph boundaries (batch is sorted). searchsorted handles empty graphs.
    gstart = np.searchsorted(batch, np.arange(n_graphs), side="left")
    gend = np.searchsorted(batch, np.arange(n_graphs), side="right")
    gsz = gend - gstart

    # graphs 8k..8k+7 -> core k; graph-block-aligned node layout per core
    gblk = np.maximum((gsz + P - 1) // P, 1)  # blocks per graph (>=1 slot even if empty)
    nblk_core = [int(gblk[k * GPC:(k + 1) * GPC].sum()) for k in range(NCORE)]
    NBLK = max(nblk_core)
    NMAXP = NBLK * P

    # node global id -> (core, local slot). graph g local base within its core:
    loc_base = np.zeros(n_graphs, np.int64)
    for g in range(n_graphs):
        k = g // GPC
        if g % GPC == 0:
            loc_base[g] = 0
        else:
            loc_base[g] = loc_base[g - 1] + gblk[g - 1] * P
    node_core = batch // GPC
    node_loc = loc_base[batch] + (np.arange(N) - gstart[batch])
    # partition-major table storage: row = core*NMAXP + (loc%128)*NBLK + loc//128
    # (lets shard/table writes be one contiguous-per-partition DMA)
    node_row = node_core * NMAXP + (node_loc % P) * NBLK + node_loc // P

    HALF = 4 * NMAXP
    assert HALF + 4 * NMAXP <= 8 * NMAXP and HALF < 32768, f"HALF={HALF}"

    # edges -> owner core by dst
    e_core = node_core[dst_g]
    e_dstloc = node_loc[dst_g]
    e_blk = e_dstloc // P
    e_dl = e_dstloc % P
    e_row = node_row[src_g]
    e_half = (e_row >= HALF).astype(np.int64)

    # per (core, block, half) edge lists; compute CPA/CPB
    cnts = np.zeros((NCORE, NBLK, 2), np.int64)
    np.add.at(cnts, (e_core, e_blk, e_half), 1)
    CPA = int((cnts[:, :, 0].max() + P - 1) // P)
    CPB = int((cnts[:, :, 1].max() + P - 1) // P)
    CPT = CPA + CPB

    # fill per-core buffers
    # region A slots: block b chunks [b*CPA,(b+1)*CPA); region B after NBLK*CPA
    nchunk = NBLK * CPT
    idx_all = np.zeros((NCORE, nchunk * P), np.int16)
    dstloc_all = np.full((NCORE, nchunk * P), -1.0, np.float32)

    order = np.lexsort((e_half, e_blk, e_core))
    so_core, so_blk, so_half = e_core[order], e_blk[order], e_half[order]
    so_row, so_dl = e_row[order], e_dl[order]
    # positions within each (core, blk, half) run
    key = (so_core * NBLK + so_blk) * 2 + so_half
    runstart = np.r_[0, np.flatnonzero(np.diff(key)) + 1]
    runid = np.zeros(E, np.int64)
    runid[runstart[1:]] = 1
    runid = np.cumsum(runid)
    pos_in_run = np.arange(E) - runstart[runid]

    baseA = (so_blk * CPA) * P
    baseB = (NBLK * CPA + so_blk * CPB) * P
    slot = np.where(so_half == 0, baseA, baseB) + pos_in_run
    idxv = np.where(so_half == 0, so_row, so_row - HALF).astype(np.int16)
    idx_all[so_core, slot] = idxv
    dstloc_all[so_core, slot] = so_dl

    def wrap16(v):  # [n] -> [128, n/16]: idx[i%16, i//16] tiled 8x
        a = v.reshape(-1, 16).T
        return np.tile(a, (8, 1)).copy()

    # pooling masks
    gonehot = np.zeros((NCORE, NBLK * P, GPC), np.float32)
    gmask = np.zeros((NCORE, GPC, NBLK), np.float32)
    for g in range(n_graphs):
        k, gl = g // GPC, g % GPC
        b0 = loc_base[g] // P
        gmask[k, gl, b0:b0 + gblk[g]] = 1.0
        gonehot[k, loc_base[g]:loc_base[g] + gsz[g], gl] = 1.0

    cores = []
    for k in range(NCORE):
        xk = np.zeros((NMAXP, 2), np.float32)
        # scatter real nodes into padded slots
        sel = node_core == np.int64(k)
        xk[node_loc[sel]] = np.asarray(x)[sel]
        # dstloc per chunk-slot per partition, region order [nchunk, 128]
        dl = dstloc_all[k].reshape(nchunk, P)
        # block order: [NBLK, CPT] chunk slots: b's A-chunks then B-chunks
        blk_slots = np.concatenate([
            (np.arange(NBLK)[:, None] * CPA + np.arange(CPA)[None, :]),
            (NBLK * CPA + np.arange(NBLK)[:, None] * CPB + np.arange(CPB)[None, :]),
        ], axis=1)  # [NBLK, CPT]
        dl_blk = dl[blk_slots]                                   # [NBLK, CPT, 128]
        cores.append(dict(
            xT=np.ascontiguousarray(xk.T),                       # [2, NMAXP] f32
            idx=wrap16(idx_all[k]),                              # [128, nchunk*8] i16
            dstloc=np.ascontiguousarray(
                dl.T).astype(ml_dtypes.bfloat16),                # [128, nchunk]
            dstloc_blk=np.ascontiguousarray(
                dl_blk.transpose(2, 0, 1)).astype(ml_dtypes.bfloat16),  # [128, NBLK, CPT]
            gonehot=np.ascontiguousarray(
                gonehot[k].reshape(NBLK, P, GPC).transpose(1, 0, 2)).astype(np.float32),  # [128, NBLK, 8]
            gmask=np.tile(gmask[k].reshape(1, GPC * NBLK), (P, 1)).astype(np.float32),  # [128, 8*NBLK]
            gvalid=np.tile((gsz[k * GPC:(k + 1) * GPC] > 0).astype(np.float32), (P, 1)),  # [128, 8]
        ))

    meta = dict(NBLK=NBLK, NMAXP=NMAXP, CPA=CPA, CPB=CPB, CPT=CPT, HALF=HALF,
                nchunk=nchunk, gsz=gsz, cores=cores)
    # replicated full xT (bf16) for local layer-1 table build: [16, NMAXP]
    meta["xfull"] = np.concatenate([c["xT"] for c in cores], 0).astype(ml_dtypes.bfloat16)
    return meta




F32 = mybir.dt.float32
BF16 = mybir.dt.bfloat16
I16 = mybir.dt.int16
AF = mybir.ActivationFunctionType
OP = mybir.AluOpType


def build(meta, GBLK=8, H=128, OC=41, GPC=8, SINGLE_PACKET=True, ABLATE=()):
    # ABLATE: set of feature names to stub out for time attribution:
    #   "gather" -> skip dma_gather calls (garbage data, wrong results)
    #   "ag"     -> skip AllGather collectives (wrong results)
    #   "onehot" -> skip one-hot builds (wrong results)
    NBLK, NMAXP = meta["NBLK"], meta["NMAXP"]
    CPA, CPB, HALF = meta["CPA"], meta["CPB"], meta["HALF"]
    CPT = CPA + CPB
    NCH = meta["nchunk"]
    assert NCH == NBLK * CPT
    ACH = NBLK * CPA  # chunks in region A
    NCORE = 8
    NTAB = NCORE * NMAXP

    nc = bacc.Bacc(None, target_bir_lowering=False)

    # ---- IO ----
    xT_d = nc.dram_tensor("xT", [2, NMAXP], BF16, kind="ExternalInput")
    idx_d = nc.dram_tensor("idx", [128, NCH * 8], I16, kind="ExternalInput")
    dstloc_d = nc.dram_tensor("dstloc", [128, NBLK, CPT], BF16, kind="ExternalInput")
    colidx_d = nc.dram_tensor("colidx", [128, 128], BF16, kind="ExternalInput")
    ident_d = nc.dram_tensor("ident", [128, 128], F32, kind="ExternalInput")
    W1_d = nc.dram_tensor("W1", [2, H], F32, kind="ExternalInput")
    W2_d = nc.dram_tensor("W2", [H, H], F32, kind="ExternalInput")
    W3_d = nc.dram_tensor("W3", [H, H], F32, kind="ExternalInput")
    Wo_d = nc.dram_tensor("Wo", [H, 2, OC], F32, kind="ExternalInput")
    bo_d = nc.dram_tensor("bo", [GPC, OC], F32, kind="ExternalInput")
    brep_d = nc.dram_tensor("brep", [128, 3 * H], F32, kind="ExternalInput")
    goh_d = nc.dram_tensor("gonehot", [128, NBLK, GPC], F32, kind="ExternalInput")
    gmask_d = nc.dram_tensor("gmask", [128, GPC * NBLK], F32, kind="ExternalInput")
    gvalid_d = nc.dram_tensor("gvalid", [128, GPC], F32, kind="ExternalInput")
    out_d = nc.dram_tensor("out", [GPC, OC], F32, kind="ExternalOutput")

    shard_int = [nc.dram_tensor(f"shard{L}", [NMAXP, H], BF16) for L in range(1, 3)]
    table1_d = nc.dram_tensor("table1", [NTAB, H], BF16)  # locally built, no AG
    table_int = [nc.dram_tensor(f"tableL{L+1}", [NTAB, H], BF16, addr_space="Shared") for L in range(1, 3)]
    dinv_own_d = nc.dram_tensor("dinv_own", [NMAXP], F32)
    dinv_full_d = nc.dram_tensor("dinv_full", [NCORE * NMAXP], F32, addr_space="Shared")
    xfull_d = nc.dram_tensor("xfull", [2 * NCORE, NMAXP], BF16, kind="ExternalInput")

    ngrp = (NBLK + GBLK - 1) // GBLK

    with tile.TileContext(nc) as tc, ExitStack() as ctx:
        const = ctx.enter_context(tc.tile_pool(name="const", bufs=1))
        resid = ctx.enter_context(tc.tile_pool(name="resid", bufs=1))
        hTp = ctx.enter_context(tc.tile_pool(name="hTp", bufs=2))
        bsp = ctx.enter_context(tc.tile_pool(name="bsp", bufs=2))
        gap = ctx.enter_context(tc.tile_pool(name="gap", bufs=2))
        gbp = ctx.enter_context(tc.tile_pool(name="gbp", bufs=2))
        ohp = ctx.enter_context(tc.tile_pool(name="ohp", bufs=3))
        wk = ctx.enter_context(tc.tile_pool(name="wk", bufs=3))
        aggps = ctx.enter_context(tc.tile_pool(name="aggps", bufs=3, space="PSUM"))
        prepps = ctx.enter_context(tc.tile_pool(name="prepps", bufs=2, space="PSUM"))
        tps = ctx.enter_context(tc.tile_pool(name="tps", bufs=1, space="PSUM"))
        poolps = ctx.enter_context(tc.tile_pool(name="poolps", bufs=1, space="PSUM"))

        nc.gpsimd.load_library(library_config.mlp)

        # ---- P0: constants ----
        def load_const(dram, shape, dt):
            t = const.tile(shape, dt, tag=dram.name)
            nc.sync.dma_start(t[:], dram[:])
            return t

        xT_t = load_const(xT_d, [2, NMAXP], BF16)
        idx_t = load_const(idx_d, [128, NCH * 8], I16)
        dstloc_t = load_const(dstloc_d, [128, NBLK, CPT], BF16)
        colidx_t = load_const(colidx_d, [128, 128], BF16)
        ident_t = load_const(ident_d, [128, 128], F32)
        W1_t = load_const(W1_d, [2, H], F32)
        W2_t = load_const(W2_d, [H, H], F32)
        W3_t = load_const(W3_d, [H, H], F32)
        Wo_t = load_const(Wo_d, [H, 2, OC], F32)
        bo_t = load_const(bo_d, [GPC, OC], F32)
        brep_t = load_const(brep_d, [128, 3 * H], F32)
        goh_t = load_const(goh_d, [128, NBLK, GPC], F32)
        gmask_t = load_const(gmask_d, [128, GPC * NBLK], F32)
        gvalid_t = load_const(gvalid_d, [128, GPC], F32)

        ones_bf = const.tile([128, 1], BF16, tag="ones_bf")
        nc.vector.memset(ones_bf[:], 1.0)
        ones_f1 = const.tile([128, 1], F32, tag="ones_f1")
        nc.vector.memset(ones_f1[:], 1.0)
        onesrow = const.tile([1, 128], F32, tag="onesrow")
        nc.vector.memset(onesrow[:], 1.0)

        dinv_t = resid.tile([128, NBLK], F32, tag="dinv")
        sbuild = resid.tile([128, NBLK, H], BF16, tag="sbuild")  # staging for shard/table writes

        oh_shared = None
        if "onehot1" in ABLATE:
            oh_shared = const.tile([128, CPT, 128], BF16, tag="oh_shared")
            cb = colidx_t[:, None, :].broadcast_to((128, CPT, 128))
            db = dstloc_t[:, 0, :, None].broadcast_to((128, CPT, 128))
            nc.vector.tensor_tensor(oh_shared[:], cb, db, OP.is_equal)
        g_shared = None
        if "gather" in ABLATE:
            g_shared = const.tile([128, max(CPA, CPB), H], BF16, tag="g_shared")
            nc.vector.memset(g_shared[:], 0.125)

        def onehot(b):
            if oh_shared is not None:
                return oh_shared
            oh = ohp.tile([128, CPT, 128], BF16, tag="oh")
            cb = colidx_t[:, None, :].broadcast_to((128, CPT, 128))
            db = dstloc_t[:, b, :, None].broadcast_to((128, CPT, 128))
            nc.vector.tensor_tensor(oh[:], cb, db, OP.is_equal)
            return oh

        # ---- P1: deg pass ----
        degsum = resid.tile([128, NBLK], F32, tag="degsum")
        for b in range(NBLK):
            oh = onehot(b)
            dp = aggps.tile([128, 128], F32, tag="agg")  # only col 0 used
            for c in range(CPT):
                nc.tensor.matmul(
                    dp[:, 0:1], oh[:, c, :], ones_bf[:],
                    start=(c == 0), stop=(c == CPT - 1),
                )
            # deg+1 into degsum column
            nc.vector.tensor_scalar(
                degsum[:, b : b + 1], dp[:, 0:1], 1.0, None, OP.add
            )
        recd = resid.tile([128, NBLK], F32, tag="recd")
        nc.vector.reciprocal(recd[:], degsum[:])
        nc.scalar.sqrt(dinv_t[:], recd[:])

        # dinv -> node-order DRAM -> AllGather (once; reused every layer)
        dvp = prepps.tile([NBLK, 128], F32, tag="prep")
        nc.tensor.transpose(dvp[:], dinv_t[:], ident_t[:])
        dvrow = wk.tile([NBLK, 128], F32, tag="dvrow")
        nc.vector.tensor_copy(dvrow[:], dvp[:])
        nc.sync.dma_start(dinv_own_d.rearrange("(b p) -> b p", p=128)[:, :], dvrow[:])
        nc.gpsimd.collective_compute(
            "AllGather", OP.bypass, replica_groups=[list(range(NCORE))],
            ins=[dinv_own_d[:]], outs=[dinv_full_d[:]],
        )

        # layer-1 table built locally: table1[k*NMAXP + b*128 + p] = (x*dinv) @ W1
        XG = 2  # blocks per x chunk
        W1b = const.tile([2, H], BF16, tag="W1b")
        nc.vector.tensor_copy(W1b[:], W1_t[:])
        for k in range(NCORE):
            for g0 in range(0, NBLK, XG):
                g1 = min(g0 + XG, NBLK)
                nb = g1 - g0
                xch = wk.tile([2, XG * 128], BF16, tag="xch")
                nc.sync.dma_start(xch[:, : nb * 128], xfull_d[2 * k : 2 * k + 2, g0 * 128 : g1 * 128])
                dch = wk.tile([2, XG * 128], F32, tag="dch")
                for pp_ in range(2):
                    nc.sync.dma_start(
                        dch[pp_ : pp_ + 1, : nb * 128],
                        dinv_full_d[k * NMAXP + g0 * 128 : k * NMAXP + g1 * 128][None, :],
                    )
                ych = wk.tile([2, XG * 128], BF16, tag="ych")
                nc.vector.tensor_tensor(ych[:, : nb * 128], xch[:, : nb * 128], dch[:, : nb * 128], OP.mult)
                for b in range(g0, g1):
                    tp1 = prepps.tile([128, H], F32, tag="prep")
                    nc.tensor.matmul(tp1[:], ych[:, (b - g0) * 128 : (b - g0 + 1) * 128], W1b[:], start=True, stop=True)
                    nc.vector.tensor_copy(sbuild[:, b, :], tp1[:])
            nc.sync.dma_start(
                table1_d.rearrange("(k p b) h -> k p (b h)", k=NCORE, b=NBLK)[k],
                sbuild[:].rearrange("p b h -> p (b h)"),
            )

        # ---- P2: layers ----
        hT_prev = None  # SBUF tile [128f, NMAXP] f32 (None => layer1 uses xT)
        h3_blocks = []  # node-major final-layer h tiles for mean pooling
        meanp = poolps.tile([128, GPC], F32, tag="meanp")
        cntp = poolps.tile([1, GPC], F32, tag="cntp")

        for L in range(3):
            W_t = (W1_t, W2_t, W3_t)[L]
            # --- prepare: hW' per node-block -> shard dram; Bstar resident ---
            bstar = bsp.tile([128, NMAXP], BF16, tag="bstar")
            for b in range(NBLK):
                pp = prepps.tile([128, H], F32, tag="prep")
                if L == 0:
                    nc.tensor.matmul(
                        pp[:], xT_t[:, b * 128 : (b + 1) * 128], W1b[:],
                        start=True, stop=True,
                    )
                else:
                    nc.tensor.matmul(
                        pp[:], hT_prev[:, b * 128 : (b + 1) * 128], W_t[:],
                        start=True, stop=True,
                    )
                t1 = wk.tile([128, H], F32, tag="t1")
                nc.vector.tensor_scalar(
                    t1[:], pp[:], dinv_t[:, b : b + 1], None, OP.mult
                )
                if L > 0:
                    nc.vector.tensor_copy(sbuild[:, b, :], t1[:])
                # bstar = t1*dinv + b_L  (bf16)
                t2 = wk.tile([128, H], F32, tag="t2")
                nc.vector.tensor_scalar(
                    t2[:], t1[:], dinv_t[:, b : b + 1], None, OP.mult
                )
                nc.vector.tensor_tensor(
                    bstar[:, b * 128 : (b + 1) * 128],
                    t2[:], brep_t[:, L * H : (L + 1) * H], OP.add,
                )

            # --- allgather (layers 2,3 only; layer-1 table is built locally) ---
            if L > 0:
                nc.sync.dma_start(
                    shard_int[L - 1].rearrange("(p b) h -> p (b h)", b=NBLK)[:, :],
                    sbuild[:].rearrange("p b h -> p (b h)"),
                )
            if "ag" not in ABLATE and L > 0:
                nc.gpsimd.collective_compute(
                    "AllGather", OP.bypass,
                    replica_groups=[list(range(NCORE))],
                    ins=[shard_int[L - 1][:]], outs=[table_int[L - 1][:]],
                )

            # --- message pass ---
            hT_next = hTp.tile([128, NMAXP], F32, tag="hT")
            for g in range(ngrp):
                b0, b1 = g * GBLK, min((g + 1) * GBLK, NBLK)
                nblks = b1 - b0
                nA, nB = nblks * CPA * 128, nblks * CPB * 128
                if "gather" in ABLATE:
                    gA = gB = None
                else:
                    gA = gap.tile([128, GBLK * CPA, H], BF16, tag="gA")
                    gB = gbp.tile([128, GBLK * CPB, H], BF16, tag="gB")
                if "gather" not in ABLATE:
                    tab = table1_d if L == 0 else table_int[L - 1]
                    nc.gpsimd.dma_gather(
                        gA[:, : nblks * CPA, :], tab[0:HALF, :],
                        idx_t[:, b0 * CPA * 8 : b1 * CPA * 8], nA, nA, H,
                        single_packet=SINGLE_PACKET,
                    )
                    nc.gpsimd.dma_gather(
                        gB[:, : nblks * CPB, :], tab[HALF:, :],
                        idx_t[:, (ACH + b0 * CPB) * 8 : (ACH + b1 * CPB) * 8], nB, nB, H,
                        single_packet=SINGLE_PACKET,
                    )
                for b in range(b0, b1):
                    oh = onehot(b)
                    ap = aggps.tile([128, H], F32, tag="agg")
                    for c in range(CPT):
                        if gA is None:
                            rhs = g_shared[:, c % max(CPA, CPB), :]
                        else:
                            rhs = (
                                gA[:, (b - b0) * CPA + c, :]
                                if c < CPA
                                else gB[:, (b - b0) * CPB + (c - CPA), :]
                            )
                        nc.tensor.matmul(
                            ap[:], oh[:, c, :], rhs,
                            start=(c == 0), stop=(c == CPT - 1),
                        )
                    # epilogue: h = tanh(ap*dinv + bstar)
                    e1 = wk.tile([128, H], F32, tag="e1")
                    nc.vector.tensor_scalar(
                        e1[:], ap[:], dinv_t[:, b : b + 1], None, OP.mult
                    )
                    e2 = wk.tile([128, H], F32, tag="e2")
                    nc.vector.tensor_tensor(
                        e2[:], e1[:], bstar[:, b * 128 : (b + 1) * 128], OP.add
                    )
                    hblk = wk.tile([128, H], F32, tag="hblk")
                    nc.scalar.activation(hblk[:], e2[:], AF.Tanh)
                    if L == 2:
                        # mean-pool and count matmuls (accumulate over all blocks)
                        nc.tensor.matmul(
                            meanp[:], hblk[:], goh_t[:, b, :],
                            start=(b == 0), stop=(b == NBLK - 1),
                        )
                        nc.tensor.matmul(
                            cntp[:], ones_f1[:], goh_t[:, b, :],
                            start=(b == 0), stop=(b == NBLK - 1),
                        )
                    # transpose to hT_next
                    tp = tps.tile([128, H], F32, tag="tp")
                    nc.tensor.transpose(tp[:], hblk[:], ident_t[:])
                    nc.vector.tensor_copy(hT_next[:, b * 128 : (b + 1) * 128], tp[:])
            hT_prev = hT_next

        # ---- P3: pooling + head ----
        h3T = hT_prev
        # block-partial max -> P [128f, NBLK], then +2, masked max per graph
        pmax = resid.tile([128, NBLK], F32, tag="pmax")
        for b in range(NBLK):
            nc.vector.tensor_reduce(
                pmax[:, b : b + 1], h3T[:, b * 128 : (b + 1) * 128],
                mybir.AxisListType.X, OP.max,
            )
        p2 = resid.tile([128, NBLK], F32, tag="p2")
        nc.vector.tensor_scalar(p2[:], pmax[:], 2.0, None, OP.add)
        mxT = resid.tile([128, GPC], F32, tag="mxT")
        for gph in range(GPC):
            mg = wk.tile([128, NBLK], F32, tag="mg")
            nc.vector.tensor_tensor(
                mg[:], p2[:], gmask_t[:, gph * NBLK : (gph + 1) * NBLK], OP.mult
            )
            nc.vector.tensor_reduce(
                mxT[:, gph : gph + 1], mg[:], mybir.AxisListType.X, OP.max
            )
        mxT2a = resid.tile([128, GPC], F32, tag="mxT2a")
        nc.vector.tensor_scalar(mxT2a[:], mxT[:], 2.0, None, OP.subtract)
        mxT2 = resid.tile([128, GPC], F32, tag="mxT2")
        nc.vector.tensor_tensor(mxT2[:], mxT2a[:], gvalid_t[:], OP.mult)

        # mean = meanp / max(cnt,1): rec=1/max(cnt,1) [1,8] -> replicate via matmul
        cnt_sb = wk.tile([1, GPC], F32, tag="cnt_sb")
        nc.vector.tensor_scalar(cnt_sb[:], cntp[:], 1.0, None, OP.max)
        rec_sb = wk.tile([1, GPC], F32, tag="rec_sb")
        nc.vector.reciprocal(rec_sb[:], cnt_sb[:])
        recrep = prepps.tile([128, GPC], F32, tag="prep")
        nc.tensor.matmul(recrep[:], onesrow[:], rec_sb[:], start=True, stop=True)
        recrep_sb = wk.tile([128, GPC], F32, tag="recrep_sb")
        nc.vector.tensor_copy(recrep_sb[:], recrep[:])
        meanT = wk.tile([128, GPC], F32, tag="meanT")
        nc.vector.tensor_tensor(meanT[:], meanp[:], recrep_sb[:], OP.mult)

        # head: out[8,41] = tanh(mxT2.T@Wo1 + meanT.T@Wo2 + bo)
        headp = prepps.tile([GPC, OC], F32, tag="prep")
        nc.tensor.matmul(headp[:], mxT2[:], Wo_t[:, 0, :], start=True, stop=False)
        nc.tensor.matmul(headp[:], meanT[:], Wo_t[:, 1, :], start=False, stop=True)
        hsum = wk.tile([GPC, OC], F32, tag="hsum")
        nc.vector.tensor_tensor(hsum[:], headp[:], bo_t[:], OP.add)
        ofin = wk.tile([GPC, OC], F32, tag="ofin")
        nc.scalar.activation(ofin[:], hsum[:], AF.Tanh)
        nc.sync.dma_start(out_d[:], ofin[:])

    nc.compile()
    return nc


def make_in_maps(meta, inputs, GPC=8, H=128, OC=41):
    """Build per-core input maps from hostprep meta + original model inputs."""
    import ml_dtypes
    colidx = np.tile(np.arange(128, dtype=np.float32), (128, 1)).astype(ml_dtypes.bfloat16)
    ident = np.eye(128, dtype=np.float32)
    brep = np.tile(
        np.concatenate([np.asarray(inputs[k], np.float32) for k in ("b1", "b2", "b3")]),
        (128, 1),
    ).astype(np.float32)
    bo_t = np.tile(np.asarray(inputs["bo"], np.float32), (GPC, 1))
    NBLK, CPT = meta["NBLK"], meta["CPT"]
    maps = []
    for c in meta["cores"]:
        maps.append({
            "xT": np.asarray(c["xT"]).astype(ml_dtypes.bfloat16),
            "xfull": np.asarray(meta["xfull"]),
            "idx": c["idx"],
            # dstloc arrives [128, nchunk] region-ordered; reorder to [128, NBLK, CPT]
            "dstloc": c["dstloc_blk"],
            "colidx": colidx,
            "ident": ident,
            "W1": np.asarray(inputs["W1"], np.float32),
            "W2": np.asarray(inputs["W2"], np.float32),
            "W3": np.asarray(inputs["W3"], np.float32),
            "Wo": np.ascontiguousarray(
                np.stack([np.asarray(inputs["Wo"], np.float32)[:H],
                          np.asarray(inputs["Wo"], np.float32)[H:]], axis=1)),
            "bo": bo_t,
            "brep": brep,
            "gonehot": np.asarray(c["gonehot"], np.float32),
            "gmask": np.asarray(c["gmask"], np.float32),
            "gvalid": np.asarray(c["gvalid"], np.float32),
        })
    return maps


_CACHE = {}


def kernel(x, edge_index, batch, W1, b1, W2, b2, W3, b3, Wo, bo):
    x = np.asarray(x, np.float32)
    edge_index = np.asarray(edge_index)
    batch = np.asarray(batch)
    meta = prep(x, edge_index, batch, 64)
    key = (meta["NBLK"], meta["CPA"], meta["CPB"])
    if key not in _CACHE:
        _CACHE[key] = build(meta, GBLK=6, SINGLE_PACKET=False)
    nc = _CACHE[key]
    inputs = dict(W1=W1, b1=b1, W2=W2, b2=b2, W3=W3, b3=b3, Wo=Wo, bo=bo)
    in_maps = make_in_maps(meta, inputs)
    res = run_bass_kernel_spmd(nc, in_maps, core_ids=list(range(8)), trace=False)
    out = np.concatenate([res.results[k]["out"] for k in range(8)], 0)
    return np.ascontiguousarray(out, dtype=np.float32)



# revision 7
# speedup vs baseline: 1.0008x; 1.0008x over previous
"""Self-contained Trainium2 Bass kernel for nn_GCNMagnetModel (3-layer GCN,
N=50000 nodes, E=600000 edges, H=128, 64 graphs, 8 NeuronCores, SPMD 1 NEFF).

Sharding (hardcoded strategy): nodes/edges sharded across 8 cores by graph id
(graphs 8k..8k+7 -> core k; graph-block-aligned node layout so both pools are
core-local). Message passing per dst-block of 128 nodes via one-hot matmuls on
the PE over dma_gather'd rows of a per-layer bf16 table of
hW' = (h @ W) * rsqrt(deg) in partition-major storage (row = k*NMAXP +
(loc%128)*NBLK + loc//128, so shard/table writes are single large
contiguous-per-partition DMAs). GCN norm is separable: dinv_src folded into
the table, dinv_dst applied per dst partition. The layer-1 table is built
locally on every core from replicated x plus one small dinv AllGather (x has
only 2 features), avoiding one of the three large table AllGathers. Degree
counting, rsqrt, all matmuls, tanh, and max/mean pools run on device; the
host only computes index layout (sorting, padding, local renumbering, masks).

kernel(**inputs) -> [64, 41] float32.
"""
import numpy as np
import ml_dtypes
from contextlib import ExitStack

import concourse.tile as tile
import concourse.mybir as mybir
from concourse import bacc
from concourse import library_config
from concourse.bass_utils import run_bass_kernel_spmd

"""(inlined hostprep) Host-side sharding/layout prep for the GCN kernel, plus a numpy emulator
of the exact device dataflow (bf16 table, one-hot matmuls, per-block pipeline)
to validate index bookkeeping and predict accuracy at full scale.

Host only does index/layout manipulation here (sorting, partitioning,
padding, local renumbering, masks, replication of given constants). All FP
math on tensor values happens on-device (emulated in emu_device()).
"""
import numpy as np
import ml_dtypes

NCORE = 8
P = 128
GPC = 8  # graphs per core


def prep(x, edge_index, batch, n_graphs=64):
    N = x.shape[0]
    batch = np.asarray(batch)
    src_g, dst_g = np.asarray(edge_index[0]), np.asarray(edge_index[1])
    E = src_g.shape[0]

    # graph boundaries (batch is sorted). searchsorted handles empty graphs.
    gstart = np.searchsorted(batch, np.arange(n_graphs), side="left")
    gend = np.searchsorted(batch, np.arange(n_graphs), side="right")
    gsz = gend - gstart

    # graphs 8k..8k+7 -> core k; graph-block-aligned node layout per core
    gblk = np.maximum((gsz + P - 1) // P, 1)  # blocks per graph (>=1 slot even if empty)
    nblk_core = [int(gblk[k * GPC:(k + 1) * GPC].sum()) for k in range(NCORE)]
    NBLK = max(nblk_core)
    NMAXP = NBLK * P

    # node global id -> (core, local slot). graph g local base within its core:
    loc_base = np.zeros(n_graphs, np.int64)
    for g in range(n_graphs):
        k = g // GPC
        if g % GPC == 0:
            loc_base[g] = 0
        else:
            loc_base[g] = loc_base[g - 1] + gblk[g - 1] * P
    node_core = batch // GPC
    node_loc = loc_base[batch] + (np.arange(N) - gstart[batch])
    # partition-major table storage: row = core*NMAXP + (loc%128)*NBLK + loc//128
    # (lets shard/table writes be one contiguous-per-partition DMA)
    node_row = node_core * NMAXP + (node_loc % P) * NBLK + node_loc // P

    HALF = 4 * NMAXP
    assert HALF + 4 * NMAXP <= 8 * NMAXP and HALF < 32768, f"HALF={HALF}"

    # edges -> owner core by dst
    e_core = node_core[dst_g]
    e_dstloc = node_loc[dst_g]
    e_blk = e_dstloc // P
    e_dl = e_dstloc % P
    e_row = node_row[src_g]
    e_half = (e_row >= HALF).astype(np.int64)

    # per (core, block, half) edge lists; compute CPA/CPB
    cnts = np.zeros((NCORE, NBLK, 2), np.int64)
    np.add.at(cnts, (e_core, e_blk, e_half), 1)
    CPA = int((cnts[:, :, 0].max() + P - 1) // P)
    CPB = int((cnts[:, :, 1].max() + P - 1) // P)
    CPT = CPA + CPB

    # fill per-core buffers
    # region A slots: block b chunks [b*CPA,(b+1)*CPA); region B after NBLK*CPA
    nchunk = NBLK * CPT
    idx_all = np.zeros((NCORE, nchunk * P), np.int16)
    dstloc_all = np.full((NCORE, nchunk * P), -1.0, np.float32)

    order = np.lexsort((e_half, e_blk, e_core))
    so_core, so_blk, so_half = e_core[order], e_blk[order], e_half[order]
    so_row, so_dl = e_row[order], e_dl[order]
    # positions within each (core, blk, half) run
    key = (so_core * NBLK + so_blk) * 2 + so_half
    runstart = np.r_[0, np.flatnonzero(np.diff(key)) + 1]
    runid = np.zeros(E, np.int64)
    runid[runstart[1:]] = 1
    runid = np.cumsum(runid)
    pos_in_run = np.arange(E) - runstart[runid]

    baseA = (so_blk * CPA) * P
    baseB = (NBLK * CPA + so_blk * CPB) * P
    slot = np.where(so_half == 0, baseA, baseB) + pos_in_run
    idxv = np.where(so_half == 0, so_row, so_row - HALF).astype(np.int16)
    idx_all[so_core, slot] = idxv
    dstloc_all[so_core, slot] = so_dl

    def wrap16(v):  # [n] -> [128, n/16]: idx[i%16, i//16] tiled 8x
        a = v.reshape(-1, 16).T
        return np.tile(a, (8, 1)).copy()

    # pooling masks
    gonehot = np.zeros((NCORE, NBLK * P, GPC), np.float32)
    gmask = np.zeros((NCORE, GPC, NBLK), np.float32)
    for g in range(n_graphs):
        k, gl = g // GPC, g % GPC
        b0 = loc_base[g] // P
        gmask[k, gl, b0:b0 + gblk[g]] = 1.0
        gonehot[k, loc_base[g]:loc_base[g] + gsz[g], gl] = 1.0

    cores = []
    for k in range(NCORE):
        xk = np.zeros((NMAXP, 2), np.float32)
        # scatter real nodes into padded slots
        sel = node_core == np.int64(k)
        xk[node_loc[sel]] = np.asarray(x)[sel]
        # dstloc per chunk-slot per partition, region order [nchunk, 128]
        dl = dstloc_all[k].reshape(nchunk, P)
        # block order: [NBLK, CPT] chunk slots: b's A-chunks then B-chunks
        blk_slots = np.concatenate([
            (np.arange(NBLK)[:, None] * CPA + np.arange(CPA)[None, :]),
            (NBLK * CPA + np.arange(NBLK)[:, None] * CPB + np.arange(CPB)[None, :]),
        ], axis=1)  # [NBLK, CPT]
        dl_blk = dl[blk_slots]                                   # [NBLK, CPT, 128]
        cores.append(dict(
            xT=np.ascontiguousarray(xk.T),                       # [2, NMAXP] f32
            idx=wrap16(idx_all[k]),                              # [128, nchunk*8] i16
            dstloc=np.ascontiguousarray(
                dl.T).astype(ml_dtypes.bfloat16),                # [128, nchunk]
            dstloc_blk=np.ascontiguousarray(
                dl_blk.transpose(2, 0, 1)).astype(ml_dtypes.bfloat16),  # [128, NBLK, CPT]
            gonehot=np.ascontiguousarray(
                gonehot[k].reshape(NBLK, P, GPC).transpose(1, 0, 2)).astype(np.float32),  # [128, NBLK, 8]
            gmask=np.tile(gmask[k].reshape(1, GPC * NBLK), (P, 1)).astype(np.float32),  # [128, 8*NBLK]
            gvalid=np.tile((gsz[k * GPC:(k + 1) * GPC] > 0).astype(np.float32), (P, 1)),  # [128, 8]
        ))

    meta = dict(NBLK=NBLK, NMAXP=NMAXP, CPA=CPA, CPB=CPB, CPT=CPT, HALF=HALF,
                nchunk=nchunk, gsz=gsz, cores=cores)
    # replicated full xT (bf16) for local layer-1 table build: [16, NMAXP]
    meta["xfull"] = np.concatenate([c["xT"] for c in cores], 0).astype(ml_dtypes.bfloat16)
    return meta




F32 = mybir.dt.float32
BF16 = mybir.dt.bfloat16
I16 = mybir.dt.int16
AF = mybir.ActivationFunctionType
OP = mybir.AluOpType


def build(meta, GBLK=8, H=128, OC=41, GPC=8, SINGLE_PACKET=True, ABLATE=()):
    # ABLATE: set of feature names to stub out for time attribution:
    #   "gather" -> skip dma_gather calls (garbage data, wrong results)
    #   "ag"     -> skip AllGather collectives (wrong results)
    #   "onehot" -> skip one-hot builds (wrong results)
    NBLK, NMAXP = meta["NBLK"], meta["NMAXP"]
    CPA, CPB, HALF = meta["CPA"], meta["CPB"], meta["HALF"]
    CPT = CPA + CPB
    NCH = meta["nchunk"]
    assert NCH == NBLK * CPT
    ACH = NBLK * CPA  # chunks in region A
    NCORE = 8
    NTAB = NCORE * NMAXP

    nc = bacc.Bacc(None, target_bir_lowering=False)

    # ---- IO ----
    xT_d = nc.dram_tensor("xT", [2, NMAXP], BF16, kind="ExternalInput")
    idx_d = nc.dram_tensor("idx", [128, NCH * 8], I16, kind="ExternalInput")
    dstloc_d = nc.dram_tensor("dstloc", [128, NBLK, CPT], BF16, kind="ExternalInput")
    colidx_d = nc.dram_tensor("colidx", [128, 128], BF16, kind="ExternalInput")
    ident_d = nc.dram_tensor("ident", [128, 128], F32, kind="ExternalInput")
    W1_d = nc.dram_tensor("W1", [2, H], F32, kind="ExternalInput")
    W2_d = nc.dram_tensor("W2", [H, H], F32, kind="ExternalInput")
    W3_d = nc.dram_tensor("W3", [H, H], F32, kind="ExternalInput")
    Wo_d = nc.dram_tensor("Wo", [H, 2, OC], F32, kind="ExternalInput")
    bo_d = nc.dram_tensor("bo", [GPC, OC], F32, kind="ExternalInput")
    brep_d = nc.dram_tensor("brep", [128, 3 * H], F32, kind="ExternalInput")
    goh_d = nc.dram_tensor("gonehot", [128, NBLK, GPC], F32, kind="ExternalInput")
    gmask_d = nc.dram_tensor("gmask", [128, GPC * NBLK], F32, kind="ExternalInput")
    gvalid_d = nc.dram_tensor("gvalid", [128, GPC], F32, kind="ExternalInput")
    out_d = nc.dram_tensor("out", [GPC, OC], F32, kind="ExternalOutput")

    shard_int = [nc.dram_tensor(f"shard{L}", [NMAXP, H], BF16) for L in range(1, 3)]
    table1_d = nc.dram_tensor("table1", [NTAB, H], BF16)  # locally built, no AG
    table_int = [nc.dram_tensor(f"tableL{L+1}", [NTAB, H], BF16, addr_space="Shared") for L in range(1, 3)]
    dinv_own_d = nc.dram_tensor("dinv_own", [NMAXP], F32)
    dinv_full_d = nc.dram_tensor("dinv_full", [NCORE * NMAXP], F32, addr_space="Shared")
    xfull_d = nc.dram_tensor("xfull", [2 * NCORE, NMAXP], BF16, kind="ExternalInput")

    ngrp = (NBLK + GBLK - 1) // GBLK

    with tile.TileContext(nc) as tc, ExitStack() as ctx:
        const = ctx.enter_context(tc.tile_pool(name="const", bufs=1))
        resid = ctx.enter_context(tc.tile_pool(name="resid", bufs=1))
        hTp = ctx.enter_context(tc.tile_pool(name="hTp", bufs=2))
        bsp = ctx.enter_context(tc.tile_pool(name="bsp", bufs=2))
        gap = ctx.enter_context(tc.tile_pool(name="gap", bufs=2))
        gbp = ctx.enter_context(tc.tile_pool(name="gbp", bufs=2))
        ohp = ctx.enter_context(tc.tile_pool(name="ohp", bufs=3))
        wk = ctx.enter_context(tc.tile_pool(name="wk", bufs=3))
        aggps = ctx.enter_context(tc.tile_pool(name="aggps", bufs=3, space="PSUM"))
        prepps = ctx.enter_context(tc.tile_pool(name="prepps", bufs=2, space="PSUM"))
        tps = ctx.enter_context(tc.tile_pool(name="tps", bufs=1, space="PSUM"))
        poolps = ctx.enter_context(tc.tile_pool(name="poolps", bufs=1, space="PSUM"))

        nc.gpsimd.load_library(library_config.mlp)

        # ---- P0: constants ----
        def load_const(dram, shape, dt):
            t = const.tile(shape, dt, tag=dram.name)
            nc.sync.dma_start(t[:], dram[:])
            return t

        xT_t = load_const(xT_d, [2, NMAXP], BF16)
        idx_t = load_const(idx_d, [128, NCH * 8], I16)
        dstloc_t = load_const(dstloc_d, [128, NBLK, CPT], BF16)
        colidx_t = load_const(colidx_d, [128, 128], BF16)
        ident_t = load_const(ident_d, [128, 128], F32)
        W1_t = load_const(W1_d, [2, H], F32)
        W2_t = load_const(W2_d, [H, H], F32)
        W3_t = load_const(W3_d, [H, H], F32)
        Wo_t = load_const(Wo_d, [H, 2, OC], F32)
        bo_t = load_const(bo_d, [GPC, OC], F32)
        brep_t = load_const(brep_d, [128, 3 * H], F32)
        goh_t = load_const(goh_d, [128, NBLK, GPC], F32)
        gmask_t = load_const(gmask_d, [128, GPC * NBLK], F32)
        gvalid_t = load_const(gvalid_d, [128, GPC], F32)

        ones_bf = const.tile([128, 1], BF16, tag="ones_bf")
        nc.vector.memset(ones_bf[:], 1.0)
        ones_f1 = const.tile([128, 1], F32, tag="ones_f1")
        nc.vector.memset(ones_f1[:], 1.0)
        onesrow = const.tile([1, 128], F32, tag="onesrow")
        nc.vector.memset(onesrow[:], 1.0)

        dinv_t = resid.tile([128, NBLK], F32, tag="dinv")
        sbuild = resid.tile([128, NBLK, H], BF16, tag="sbuild")  # staging for shard/table writes

        oh_shared = None
        if "onehot1" in ABLATE:
            oh_shared = const.tile([128, CPT, 128], BF16, tag="oh_shared")
            cb = colidx_t[:, None, :].broadcast_to((128, CPT, 128))
            db = dstloc_t[:, 0, :, None].broadcast_to((128, CPT, 128))
            nc.vector.tensor_tensor(oh_shared[:], cb, db, OP.is_equal)
        g_shared = None
        if "gather" in ABLATE:
            g_shared = const.tile([128, max(CPA, CPB), H], BF16, tag="g_shared")
            nc.vector.memset(g_shared[:], 0.125)

        def onehot(b):
            if oh_shared is not None:
                return oh_shared
            oh = ohp.tile([128, CPT, 128], BF16, tag="oh")
            cb = colidx_t[:, None, :].broadcast_to((128, CPT, 128))
            db = dstloc_t[:, b, :, None].broadcast_to((128, CPT, 128))
            nc.vector.tensor_tensor(oh[:], cb, db, OP.is_equal)
            return oh

        # ---- P1: deg pass ----
        degsum = resid.tile([128, NBLK], F32, tag="degsum")
        if "deg" in ABLATE:
            nc.vector.memset(degsum[:], 9.0)
        else:
            for b in range(NBLK):
                oh = onehot(b)
                dp = aggps.tile([128, 128], F32, tag="agg")  # only col 0 used
                for c in range(CPT):
                    nc.tensor.matmul(
                        dp[:, 0:1], oh[:, c, :], ones_bf[:],
                        start=(c == 0), stop=(c == CPT - 1),
                    )
                # deg+1 into degsum column
                nc.vector.tensor_scalar(
                    degsum[:, b : b + 1], dp[:, 0:1], 1.0, None, OP.add
                )
        recd = resid.tile([128, NBLK], F32, tag="recd")
        nc.vector.reciprocal(recd[:], degsum[:])
        nc.scalar.sqrt(dinv_t[:], recd[:])

        # dinv -> node-order DRAM -> AllGather (once; reused every layer)
        dvp = prepps.tile([NBLK, 128], F32, tag="prep")
        nc.tensor.transpose(dvp[:], dinv_t[:], ident_t[:])
        dvrow = wk.tile([NBLK, 128], F32, tag="dvrow")
        nc.vector.tensor_copy(dvrow[:], dvp[:])
        nc.sync.dma_start(dinv_own_d.rearrange("(b p) -> b p", p=128)[:, :], dvrow[:])
        nc.gpsimd.collective_compute(
            "AllGather", OP.bypass, replica_groups=[list(range(NCORE))],
            ins=[dinv_own_d[:]], outs=[dinv_full_d[:]],
        )

        # layer-1 table built locally: table1[k*NMAXP + b*128 + p] = (x*dinv) @ W1
        XG = 2  # blocks per x chunk
        W1b = const.tile([2, H], BF16, tag="W1b")
        nc.vector.tensor_copy(W1b[:], W1_t[:])
        for k in (range(0) if "t1" in ABLATE else range(NCORE)):
            for g0 in range(0, NBLK, XG):
                g1 = min(g0 + XG, NBLK)
                nb = g1 - g0
                xch = wk.tile([2, XG * 128], BF16, tag="xch")
                nc.sync.dma_start(xch[:, : nb * 128], xfull_d[2 * k : 2 * k + 2, g0 * 128 : g1 * 128])
                dch = wk.tile([2, XG * 128], F32, tag="dch")
                for pp_ in range(2):
                    nc.sync.dma_start(
                        dch[pp_ : pp_ + 1, : nb * 128],
                        dinv_full_d[k * NMAXP + g0 * 128 : k * NMAXP + g1 * 128][None, :],
                    )
                ych = wk.tile([2, XG * 128], BF16, tag="ych")
                nc.vector.tensor_tensor(ych[:, : nb * 128], xch[:, : nb * 128], dch[:, : nb * 128], OP.mult)
                for b in range(g0, g1):
                    tp1 = prepps.tile([128, H], F32, tag="prep")
                    nc.tensor.matmul(tp1[:], ych[:, (b - g0) * 128 : (b - g0 + 1) * 128], W1b[:], start=True, stop=True)
                    nc.vector.tensor_copy(sbuild[:, b, :], tp1[:])
            nc.sync.dma_start(
                table1_d.rearrange("(k p b) h -> k p (b h)", k=NCORE, b=NBLK)[k],
                sbuild[:].rearrange("p b h -> p (b h)"),
            )

        # ---- P2: layers ----
        hT_prev = None  # SBUF tile [128f, NMAXP] f32 (None => layer1 uses xT)
        h3_blocks = []  # node-major final-layer h tiles for mean pooling
        meanp = poolps.tile([128, GPC], F32, tag="meanp")
        cntp = poolps.tile([1, GPC], F32, tag="cntp")

        for L in range(3):
            W_t = (W1_t, W2_t, W3_t)[L]
            # --- prepare: hW' per node-block -> shard dram; Bstar resident ---
            bstar = bsp.tile([128, NMAXP], BF16, tag="bstar")
            if "prep" in ABLATE:
                nc.vector.memset(bstar[:], 0.0)
                nc.vector.memset(sbuild[:].rearrange("p b h -> p (b h)"), 0.0)
            for b in (range(0) if "prep" in ABLATE else range(NBLK)):
                pp = prepps.tile([128, H], F32, tag="prep")
                if L == 0:
                    nc.tensor.matmul(
                        pp[:], xT_t[:, b * 128 : (b + 1) * 128], W1b[:],
                        start=True, stop=True,
                    )
                else:
                    nc.tensor.matmul(
                        pp[:], hT_prev[:, b * 128 : (b + 1) * 128], W_t[:],
                        start=True, stop=True,
                    )
                t1 = wk.tile([128, H], F32, tag="t1")
                nc.vector.tensor_scalar(
                    t1[:], pp[:], dinv_t[:, b : b + 1], None, OP.mult
                )
                if L > 0:
                    nc.vector.tensor_copy(sbuild[:, b, :], t1[:])
                # bstar = t1*dinv + b_L  (bf16)
                t2 = wk.tile([128, H], F32, tag="t2")
                nc.vector.tensor_scalar(
                    t2[:], t1[:], dinv_t[:, b : b + 1], None, OP.mult
                )
                nc.vector.tensor_tensor(
                    bstar[:, b * 128 : (b + 1) * 128],
                    t2[:], brep_t[:, L * H : (L + 1) * H], OP.add,
                )

            # --- allgather (layers 2,3 only; layer-1 table is built locally) ---
            if L > 0:
                nc.sync.dma_start(
                    shard_int[L - 1].rearrange("(p b) h -> p (b h)", b=NBLK)[:, :],
                    sbuild[:].rearrange("p b h -> p (b h)"),
                )
            if "ag" not in ABLATE and L > 0:
                nc.gpsimd.collective_compute(
                    "AllGather", OP.bypass,
                    replica_groups=[list(range(NCORE))],
                    ins=[shard_int[L - 1][:]], outs=[table_int[L - 1][:]],
                )

            # --- message pass ---
            hT_next = hTp.tile([128, NMAXP], F32, tag="hT")
            for g in range(ngrp):
                b0, b1 = g * GBLK, min((g + 1) * GBLK, NBLK)
                nblks = b1 - b0
                nA, nB = nblks * CPA * 128, nblks * CPB * 128
                if "gather" in ABLATE:
                    gA = gB = None
                else:
                    gA = gap.tile([128, GBLK * CPA, H], BF16, tag="gA")
                    gB = gbp.tile([128, GBLK * CPB, H], BF16, tag="gB")
                if "gather" not in ABLATE:
                    tab = table1_d if L == 0 else table_int[L - 1]
                    nc.gpsimd.dma_gather(
                        gA[:, : nblks * CPA, :], tab[0:HALF, :],
                        idx_t[:, b0 * CPA * 8 : b1 * CPA * 8], nA, nA, H,
                        single_packet=SINGLE_PACKET,
                    )
                    nc.gpsimd.dma_gather(
                        gB[:, : nblks * CPB, :], tab[HALF:, :],
                        idx_t[:, (ACH + b0 * CPB) * 8 : (ACH + b1 * CPB) * 8], nB, nB, H,
                        single_packet=SINGLE_PACKET,
                    )
                for b in range(b0, b1):
                    ap = aggps.tile([128, H], F32, tag="agg")
                    if "msgmm" in ABLATE:
                        nc.vector.memset(ap[:], 0.0)
                    else:
                        oh = onehot(b)
                        for c in range(CPT):
                            if gA is None:
                                rhs = g_shared[:, c % max(CPA, CPB), :]
                            else:
                                rhs = (
                                    gA[:, (b - b0) * CPA + c, :]
                                    if c < CPA
                                    else gB[:, (b - b0) * CPB + (c - CPA), :]
                                )
                            nc.tensor.matmul(
                                ap[:], oh[:, c, :], rhs,
                                start=(c == 0), stop=(c == CPT - 1),
                            )
                    # epilogue: h = tanh(ap*dinv + bstar)
                    hblk = wk.tile([128, H], F32, tag="hblk")
                    if "epi" in ABLATE:
                        nc.vector.tensor_copy(hblk[:], ap[:])
                    else:
                        e1 = wk.tile([128, H], F32, tag="e1")
                        nc.vector.tensor_scalar(
                            e1[:], ap[:], dinv_t[:, b : b + 1], None, OP.mult
                        )
                        e2 = wk.tile([128, H], F32, tag="e2")
                        nc.vector.tensor_tensor(
                            e2[:], e1[:], bstar[:, b * 128 : (b + 1) * 128], OP.add
                        )
                        nc.scalar.activation(hblk[:], e2[:], AF.Tanh)
                    if L == 2:
                        # mean-pool and count matmuls (accumulate over all blocks)
                        nc.tensor.matmul(
                            meanp[:], hblk[:], goh_t[:, b, :],
                            start=(b == 0), stop=(b == NBLK - 1),
                        )
                        nc.tensor.matmul(
                            cntp[:], ones_f1[:], goh_t[:, b, :],
                            start=(b == 0), stop=(b == NBLK - 1),
                        )
                    # transpose to hT_next
                    if "tpose" in ABLATE:
                        nc.vector.tensor_copy(hT_next[:, b * 128 : (b + 1) * 128], hblk[:])
                    else:
                        tp = tps.tile([128, H], F32, tag="tp")
                        nc.tensor.transpose(tp[:], hblk[:], ident_t[:])
                        nc.vector.tensor_copy(hT_next[:, b * 128 : (b + 1) * 128], tp[:])
            hT_prev = hT_next

        # ---- P3: pooling + head ----
        h3T = hT_prev
        # block-partial max -> P [128f, NBLK], then +2, masked max per graph
        pmax = resid.tile([128, NBLK], F32, tag="pmax")
        for b in range(NBLK):
            nc.vector.tensor_reduce(
                pmax[:, b : b + 1], h3T[:, b * 128 : (b + 1) * 128],
                mybir.AxisListType.X, OP.max,
            )
        p2 = resid.tile([128, NBLK], F32, tag="p2")
        nc.vector.tensor_scalar(p2[:], pmax[:], 2.0, None, OP.add)
        mxT = resid.tile([128, GPC], F32, tag="mxT")
        for gph in range(GPC):
            mg = wk.tile([128, NBLK], F32, tag="mg")
            nc.vector.tensor_tensor(
                mg[:], p2[:], gmask_t[:, gph * NBLK : (gph + 1) * NBLK], OP.mult
            )
            nc.vector.tensor_reduce(
                mxT[:, gph : gph + 1], mg[:], mybir.AxisListType.X, OP.max
            )
        mxT2a = resid.tile([128, GPC], F32, tag="mxT2a")
        nc.vector.tensor_scalar(mxT2a[:], mxT[:], 2.0, None, OP.subtract)
        mxT2 = resid.tile([128, GPC], F32, tag="mxT2")
        nc.vector.tensor_tensor(mxT2[:], mxT2a[:], gvalid_t[:], OP.mult)

        # mean = meanp / max(cnt,1): rec=1/max(cnt,1) [1,8] -> replicate via matmul
        cnt_sb = wk.tile([1, GPC], F32, tag="cnt_sb")
        nc.vector.tensor_scalar(cnt_sb[:], cntp[:], 1.0, None, OP.max)
        rec_sb = wk.tile([1, GPC], F32, tag="rec_sb")
        nc.vector.reciprocal(rec_sb[:], cnt_sb[:])
        recrep = prepps.tile([128, GPC], F32, tag="prep")
        nc.tensor.matmul(recrep[:], onesrow[:], rec_sb[:], start=True, stop=True)
        recrep_sb = wk.tile([128, GPC], F32, tag="recrep_sb")
        nc.vector.tensor_copy(recrep_sb[:], recrep[:])
        meanT = wk.tile([128, GPC], F32, tag="meanT")
        nc.vector.tensor_tensor(meanT[:], meanp[:], recrep_sb[:], OP.mult)

        # head: out[8,41] = tanh(mxT2.T@Wo1 + meanT.T@Wo2 + bo)
        headp = prepps.tile([GPC, OC], F32, tag="prep")
        nc.tensor.matmul(headp[:], mxT2[:], Wo_t[:, 0, :], start=True, stop=False)
        nc.tensor.matmul(headp[:], meanT[:], Wo_t[:, 1, :], start=False, stop=True)
        hsum = wk.tile([GPC, OC], F32, tag="hsum")
        nc.vector.tensor_tensor(hsum[:], headp[:], bo_t[:], OP.add)
        ofin = wk.tile([GPC, OC], F32, tag="ofin")
        nc.scalar.activation(ofin[:], hsum[:], AF.Tanh)
        nc.sync.dma_start(out_d[:], ofin[:])

    nc.compile()
    return nc


def make_in_maps(meta, inputs, GPC=8, H=128, OC=41):
    """Build per-core input maps from hostprep meta + original model inputs."""
    import ml_dtypes
    colidx = np.tile(np.arange(128, dtype=np.float32), (128, 1)).astype(ml_dtypes.bfloat16)
    ident = np.eye(128, dtype=np.float32)
    brep = np.tile(
        np.concatenate([np.asarray(inputs[k], np.float32) for k in ("b1", "b2", "b3")]),
        (128, 1),
    ).astype(np.float32)
    bo_t = np.tile(np.asarray(inputs["bo"], np.float32), (GPC, 1))
    NBLK, CPT = meta["NBLK"], meta["CPT"]
    maps = []
    for c in meta["cores"]:
        maps.append({
            "xT": np.asarray(c["xT"]).astype(ml_dtypes.bfloat16),
            "xfull": np.asarray(meta["xfull"]),
            "idx": c["idx"],
            # dstloc arrives [128, nchunk] region-ordered; reorder to [128, NBLK, CPT]
            "dstloc": c["dstloc_blk"],
            "colidx": colidx,
            "ident": ident,
            "W1": np.asarray(inputs["W1"], np.float32),
            "W2": np.asarray(inputs["W2"], np.float32),
            "W3": np.asarray(inputs["W3"], np.float32),
            "Wo": np.ascontiguousarray(
                np.stack([np.asarray(inputs["Wo"], np.float32)[:H],
                          np.asarray(inputs["Wo"], np.float32)[H:]], axis=1)),
            "bo": bo_t,
            "brep": brep,
            "gonehot": np.asarray(c["gonehot"], np.float32),
            "gmask": np.asarray(c["gmask"], np.float32),
            "gvalid": np.asarray(c["gvalid"], np.float32),
        })
    return maps


_CACHE = {}


def kernel(x, edge_index, batch, W1, b1, W2, b2, W3, b3, Wo, bo):
    x = np.asarray(x, np.float32)
    edge_index = np.asarray(edge_index)
    batch = np.asarray(batch)
    meta = prep(x, edge_index, batch, 64)
    key = (meta["NBLK"], meta["CPA"], meta["CPB"])
    if key not in _CACHE:
        _CACHE[key] = build(meta, GBLK=6, SINGLE_PACKET=False)
    nc = _CACHE[key]
    inputs = dict(W1=W1, b1=b1, W2=W2, b2=b2, W3=W3, b3=b3, Wo=Wo, bo=bo)
    in_maps = make_in_maps(meta, inputs)
    res = run_bass_kernel_spmd(nc, in_maps, core_ids=list(range(8)), trace=False)
    out = np.concatenate([res.results[k]["out"] for k in range(8)], 0)
    return np.ascontiguousarray(out, dtype=np.float32)



# revision 26
# speedup vs baseline: 1.0381x; 1.0373x over previous
"""Self-contained Trainium2 Bass kernel for nn_GCNMagnetModel (3-layer GCN,
N=50000 nodes, E=600000 edges, H=128, 64 graphs, 8 NeuronCores, SPMD 1 NEFF).

Sharding: nodes/edges sharded across 8 cores by graph id (graphs 8k..8k+7 ->
core k; graph-block-aligned node layout so both pools are core-local).

Dataflow (v2):
- Host computes all index layout AND integer degree counts (bincount); the
  device does rsqrt and every other FP op on tensor values.
- Layer 1 is rank-2: agg((x@W1)*dinv) == agg(x*dinv) @ W1, so layer-1 message
  passing gathers 2-wide u-rows (4B) instead of 128-wide table rows; the
  layer-1 table build and its AllGather disappear.
- Layers 2/3 gather bf16 table rows of t1 = (h@W)*dinv_src from an
  AllGather'd table; per dst-block-of-128 segment-sum via one-hot matmuls.
- The GCN self-loop term is one identity matmul from the SBUF-resident
  sbuild (t1) tile per block - no separate bstar machinery.
- Layers 1/2 aggregate in TRANSPOSED orientation (aggT[H,d] with the gathered
  chunk as the stationary operand) so h feeds the next layer's prepare matmul
  with no PE transpose; prepare is fused into the same block iteration, so no
  resident hT buffer exists. Layer 3 aggregates node-major for pooling.
- Per-(block,half) chunk counts are the max over the 8 cores (SPMD shapes)
  instead of a global max, roughly halving gather volume and one-hot builds.

kernel(**inputs) -> [64, 41] float32.
"""
import numpy as np
import ml_dtypes
from contextlib import ExitStack

import concourse.tile as tile
import concourse.mybir as mybir
from concourse import bacc
from concourse import library_config
from concourse.bass_utils import run_bass_kernel_spmd

NCORE = 8
P = 128
GPC = 8  # graphs per core
H = 128
OC = 41

F32 = mybir.dt.float32
BF16 = mybir.dt.bfloat16
I16 = mybir.dt.int16
AF = mybir.ActivationFunctionType
OP = mybir.AluOpType


def wrap16(v):  # [n] -> [128, n/16]: idx[i%16, i//16] tiled 8x
    a = v.reshape(-1, 16).T
    return np.tile(a, (8, 1)).copy()


def prep(x, edge_index, batch, n_graphs=64):
    N = x.shape[0]
    x = np.asarray(x, np.float32)
    batch = np.asarray(batch)
    src_g, dst_g = np.asarray(edge_index[0]), np.asarray(edge_index[1])
    E = src_g.shape[0]

    gstart = np.searchsorted(batch, np.arange(n_graphs), side="left")
    gend = np.searchsorted(batch, np.arange(n_graphs), side="right")
    gsz = gend - gstart

    gblk = np.maximum((gsz + P - 1) // P, 1)
    nblk_core = [int(gblk[k * GPC:(k + 1) * GPC].sum()) for k in range(NCORE)]
    NBLK = max(nblk_core)
    NMAXP = NBLK * P

    loc_base = np.zeros(n_graphs, np.int64)
    for g in range(n_graphs):
        if g % GPC == 0:
            loc_base[g] = 0
        else:
            loc_base[g] = loc_base[g - 1] + gblk[g - 1] * P
    node_core = batch // GPC
    node_loc = loc_base[batch] + (np.arange(N) - gstart[batch])
    node_row = node_core * NMAXP + (node_loc % P) * NBLK + node_loc // P

    HALF = 4 * NMAXP
    assert HALF < 32768, f"HALF={HALF}"

    # host degree counts (integer index work); +1 self loop
    deg = np.bincount(dst_g, minlength=N).astype(np.float32) + 1.0

    # per-core padded layouts
    # deg_pm[k][p, b] ; degrow[k][loc] ; x node-major interleaved
    deg_pm = np.ones((NCORE, P, NBLK), np.float32)
    degrow = np.ones((NCORE, NMAXP), np.float32)
    xnm2 = np.zeros((NCORE, P, NBLK * 2), np.float32)
    pidx = (node_loc % P).astype(np.int64)
    bidx = (node_loc // P).astype(np.int64)
    deg_pm[node_core, pidx, bidx] = deg
    degrow[node_core, node_loc] = deg
    xnm2[node_core, pidx, bidx * 2] = x[:, 0]
    xnm2[node_core, pidx, bidx * 2 + 1] = x[:, 1]

    # edges -> (core, blk, half); chunk counts = per-(blk,half) max over cores
    e_core = node_core[dst_g]
    e_dstloc = node_loc[dst_g]
    e_blk = e_dstloc // P
    e_dl = (e_dstloc % P).astype(np.float32)
    e_row = node_row[src_g]
    e_half = (e_row >= HALF).astype(np.int64)

    cnts = np.zeros((NCORE, NBLK, 2), np.int64)
    np.add.at(cnts, (e_core, e_blk, e_half), 1)
    cp = (cnts.max(axis=0) + P - 1) // P          # [NBLK, 2] chunks
    cpA, cpB = cp[:, 0], cp[:, 1]
    offA = np.r_[0, np.cumsum(cpA)]               # [NBLK+1]
    offB = np.r_[0, np.cumsum(cpB)]
    NCHA, NCHB = int(offA[-1]), int(offB[-1])

    # slot assignment: sort edges by (core, half, blk), fill runs
    order = np.lexsort((e_blk, e_half, e_core))
    so_core, so_blk, so_half = e_core[order], e_blk[order], e_half[order]
    so_row, so_dl = e_row[order], e_dl[order]
    key = (so_core * 2 + so_half) * NBLK + so_blk
    runstart = np.r_[0, np.flatnonzero(np.diff(key)) + 1]
    runid = np.zeros(E, np.int64)
    runid[runstart[1:]] = 1
    runid = np.cumsum(runid)
    pos_in_run = np.arange(E) - runstart[runid]

    NCH = NCHA + NCHB
    idxA = np.zeros((NCORE, NCHA * P), np.int16)
    idxB = np.zeros((NCORE, NCHB * P), np.int16)
    dlA = np.full((NCORE, NCHA * P), -1.0, np.float32)
    dlB = np.full((NCORE, NCHB * P), -1.0, np.float32)
    # per-edge-slot source x and deg (layer-1 aggregates rank-2 u = x*dinv
    # directly from these, no gather): A slots then B slots
    xes = np.zeros((NCORE, NCH * P, 2), np.float32)
    deges = np.ones((NCORE, NCH * P), np.float32)
    so_src = src_g[order]
    isA = so_half == 0
    slotA = offA[so_blk[isA]] * P + pos_in_run[isA]
    idxA[so_core[isA], slotA] = so_row[isA].astype(np.int16)
    dlA[so_core[isA], slotA] = so_dl[isA]
    xes[so_core[isA], slotA] = x[so_src[isA]]
    deges[so_core[isA], slotA] = deg[so_src[isA]]
    isB = ~isA
    slotB = offB[so_blk[isB]] * P + pos_in_run[isB]
    idxB[so_core[isB], slotB] = (so_row[isB] - HALF).astype(np.int16)
    dlB[so_core[isB], slotB] = so_dl[isB]
    xes[so_core[isB], NCHA * P + slotB] = x[so_src[isB]]
    deges[so_core[isB], NCHA * P + slotB] = deg[so_src[isB]]

    # pooling masks
    gonehot = np.zeros((NCORE, NBLK * P, GPC), np.float32)
    gmask = np.zeros((NCORE, GPC, NBLK), np.float32)
    for g in range(n_graphs):
        k, gl = g // GPC, g % GPC
        b0 = loc_base[g] // P
        gmask[k, gl, b0:b0 + gblk[g]] = 1.0
        gonehot[k, loc_base[g]:loc_base[g] + gsz[g], gl] = 1.0

    cores = []
    for k in range(NCORE):
        cores.append(dict(
            idxA=wrap16(idxA[k]),                                 # [128, NCHA*8] i16
            idxB=wrap16(idxB[k]),
            dlA=np.ascontiguousarray(
                dlA[k].reshape(NCHA, P).T).astype(ml_dtypes.bfloat16),  # [128, NCHA]
            dlB=np.ascontiguousarray(
                dlB[k].reshape(NCHB, P).T).astype(ml_dtypes.bfloat16),
            deg_pm=deg_pm[k],                                     # [128, NBLK] f32
            degrow_rep=np.tile(degrow[k][None, :], (P, 1)).astype(ml_dtypes.bfloat16),
            deg_pm2_own=np.repeat(deg_pm[k], 2, axis=1).astype(ml_dtypes.bfloat16),  # [128, 2*NBLK]
            xnm2_own=xnm2[k].astype(ml_dtypes.bfloat16),          # [128, 2*NBLK]
            xes=np.ascontiguousarray(
                xes[k].reshape(NCH, P, 2).transpose(1, 0, 2)).astype(ml_dtypes.bfloat16),  # [128, NCH, 2]
            deges=np.ascontiguousarray(
                deges[k].reshape(NCH, P).T).astype(ml_dtypes.bfloat16),  # [128, NCH]
            gonehot=np.ascontiguousarray(
                gonehot[k].reshape(NBLK, P, GPC).transpose(1, 0, 2)).astype(np.float32),
            gmask=np.tile(gmask[k].reshape(1, GPC * NBLK), (P, 1)).astype(np.float32),
            gvalid=np.tile((gsz[k * GPC:(k + 1) * GPC] > 0).astype(np.float32), (P, 1)),
            cntrep=np.tile(gsz[k * GPC:(k + 1) * GPC].astype(np.float32), (P, 1)),
        ))

    meta = dict(NBLK=NBLK, NMAXP=NMAXP, HALF=HALF, NCHA=NCHA, NCHB=NCHB,
                cpA=cpA.astype(int), cpB=cpB.astype(int),
                offA=offA.astype(int), offB=offB.astype(int),
                gsz=gsz, cores=cores)
    return meta


def build(meta, GBLK=6, SINGLE_PACKET=False):
    NBLK, NMAXP, HALF = meta["NBLK"], meta["NMAXP"], meta["HALF"]
    NCHA, NCHB = meta["NCHA"], meta["NCHB"]
    cpA, cpB, offA, offB = meta["cpA"], meta["cpB"], meta["offA"], meta["offB"]
    NTAB = NCORE * NMAXP
    ngrp = (NBLK + GBLK - 1) // GBLK
    groups = []
    for g in range(ngrp):
        b0, b1 = g * GBLK, min((g + 1) * GBLK, NBLK)
        groups.append((b0, b1, int(offA[b0]), int(offA[b1]), int(offB[b0]), int(offB[b1])))
    GMAXA = max(a1 - a0 for (_, _, a0, a1, _, _) in groups)
    GMAXB = max(bb1 - bb0 for (_, _, _, _, bb0, bb1) in groups)

    nc = bacc.Bacc(None, target_bir_lowering=False)

    # ---- IO ----
    idxA_d = nc.dram_tensor("idxA", [128, NCHA * 8], I16, kind="ExternalInput")
    idxB_d = nc.dram_tensor("idxB", [128, NCHB * 8], I16, kind="ExternalInput")
    dlA_d = nc.dram_tensor("dlA", [128, NCHA], BF16, kind="ExternalInput")
    dlB_d = nc.dram_tensor("dlB", [128, NCHB], BF16, kind="ExternalInput")
    colidx_d = nc.dram_tensor("colidx", [128, 128], BF16, kind="ExternalInput")
    identb_d = nc.dram_tensor("identb", [128, 128], BF16, kind="ExternalInput")
    ident_d = nc.dram_tensor("ident", [128, 128], F32, kind="ExternalInput")
    W1_d = nc.dram_tensor("W1", [2, H], F32, kind="ExternalInput")
    W2_d = nc.dram_tensor("W2", [H, H], F32, kind="ExternalInput")
    W3_d = nc.dram_tensor("W3", [H, H], F32, kind="ExternalInput")
    Wo_d = nc.dram_tensor("Wo", [H, 2, OC], F32, kind="ExternalInput")
    bo_d = nc.dram_tensor("bo", [GPC, OC], F32, kind="ExternalInput")
    bvec_d = nc.dram_tensor("bvec", [128, 2], F32, kind="ExternalInput")
    brep3_d = nc.dram_tensor("brep3", [128, H], F32, kind="ExternalInput")
    goh_d = nc.dram_tensor("gonehot", [128, NBLK, GPC], F32, kind="ExternalInput")
    gmask_d = nc.dram_tensor("gmask", [128, GPC * NBLK], F32, kind="ExternalInput")
    gvalid_d = nc.dram_tensor("gvalid", [128, GPC], F32, kind="ExternalInput")
    cntrep_d = nc.dram_tensor("cntrep", [128, GPC], F32, kind="ExternalInput")
    deg_pm_d = nc.dram_tensor("deg_pm", [128, NBLK], F32, kind="ExternalInput")
    degrow_rep_d = nc.dram_tensor("degrow_rep", [128, NMAXP], BF16, kind="ExternalInput")
    deg_pm2_own_d = nc.dram_tensor("deg_pm2_own", [128, 2 * NBLK], BF16, kind="ExternalInput")
    xnm2_own_d = nc.dram_tensor("xnm2_own", [128, 2 * NBLK], BF16, kind="ExternalInput")
    NCH = NCHA + NCHB
    xes_d = nc.dram_tensor("xes", [128, NCH, 2], BF16, kind="ExternalInput")
    deges_d = nc.dram_tensor("deges", [128, NCH], BF16, kind="ExternalInput")
    out_d = nc.dram_tensor("out", [GPC, OC], F32, kind="ExternalOutput")

    shard_int = [nc.dram_tensor(f"shard{L}", [NMAXP, H], BF16) for L in (2, 3)]
    table_int = [nc.dram_tensor(f"tableL{L}", [NTAB, H], BF16, addr_space="Shared") for L in (2, 3)]

    with tile.TileContext(nc) as tc, ExitStack() as ctx:
        const = ctx.enter_context(tc.tile_pool(name="const", bufs=1))
        resid = ctx.enter_context(tc.tile_pool(name="resid", bufs=1))
        gap = ctx.enter_context(tc.tile_pool(name="gap", bufs=2))
        gbp = ctx.enter_context(tc.tile_pool(name="gbp", bufs=2))
        ohp = ctx.enter_context(tc.tile_pool(name="ohp", bufs=2))
        wk = ctx.enter_context(tc.tile_pool(name="wk", bufs=3))
        scr = ctx.enter_context(tc.tile_pool(name="scr", bufs=1))
        aggps = ctx.enter_context(tc.tile_pool(name="aggps", bufs=3, space="PSUM"))
        prepps = ctx.enter_context(tc.tile_pool(name="prepps", bufs=2, space="PSUM"))
        tps = ctx.enter_context(tc.tile_pool(name="tps", bufs=2, space="PSUM"))
        poolps = ctx.enter_context(tc.tile_pool(name="poolps", bufs=1, space="PSUM"))

        nc.gpsimd.load_library(library_config.mlp)

        def load_const(dram, shape, dt):
            t = const.tile(shape, dt, tag=dram.name)
            nc.sync.dma_start(t[:], dram[:])
            return t

        idxA_t = load_const(idxA_d, [128, NCHA * 8], I16)
        idxB_t = load_const(idxB_d, [128, NCHB * 8], I16)
        dlA_t = load_const(dlA_d, [128, NCHA], BF16)
        dlB_t = load_const(dlB_d, [128, NCHB], BF16)
        colidx_t = load_const(colidx_d, [128, 128], BF16)
        identb_t = load_const(identb_d, [128, 128], BF16)
        ident_t = load_const(ident_d, [128, 128], F32)
        W1_t = load_const(W1_d, [2, H], F32)
        W2_t = load_const(W2_d, [H, H], F32)
        W3_t = load_const(W3_d, [H, H], F32)
        Wo_t = load_const(Wo_d, [H, 2, OC], F32)
        bo_t = load_const(bo_d, [GPC, OC], F32)
        bvec_t = load_const(bvec_d, [128, 2], F32)
        brep3_t = load_const(brep3_d, [128, H], F32)
        goh_t = load_const(goh_d, [128, NBLK, GPC], F32)
        gmask_t = load_const(gmask_d, [128, GPC * NBLK], F32)
        gvalid_t = load_const(gvalid_d, [128, GPC], F32)
        cntrep_t = load_const(cntrep_d, [128, GPC], F32)
        deg_pm_t = load_const(deg_pm_d, [128, NBLK], F32)
        degrow_rep_t = load_const(degrow_rep_d, [128, NMAXP], BF16)
        deg_pm2_own_t = load_const(deg_pm2_own_d, [128, 2 * NBLK], BF16)
        xnm2_own_t = load_const(xnm2_own_d, [128, 2 * NBLK], BF16)
        xes_t = load_const(xes_d, [128, NCH, 2], BF16)
        deges_t = load_const(deges_d, [128, NCH], BF16)

        # ---- P1: rsqrt (reciprocal+sqrt) + u-table ----
        def rsqrt(out_tile, in_tile, shape, tmp_tag, dt=F32):
            rec = scr.tile(shape, dt, tag=tmp_tag)
            with nc.allow_low_precision(reason="deg is exact in bf16; dinv tol ~0.4%"):
                nc.vector.reciprocal(rec[:], in_tile[:])
            nc.scalar.activation(out_tile[:], rec[:], AF.Sqrt)

        dinv_pm = resid.tile([128, NBLK], F32, tag="dinv_pm")
        rsqrt(dinv_pm, deg_pm_t, [128, NBLK], "r1")
        dinvrep = resid.tile([128, NMAXP], BF16, tag="dinvrep")
        rsqrt(dinvrep, degrow_rep_t, [128, NMAXP], "r2", dt=BF16)

        d2o = scr.tile([128, 2 * NBLK], BF16, tag="d2o")
        rsqrt(d2o, deg_pm2_own_t, [128, 2 * NBLK], "r3", dt=BF16)
        u_own = resid.tile([128, 2 * NBLK], BF16, tag="u_own")
        nc.vector.tensor_tensor(u_own[:], xnm2_own_t[:], d2o[:], OP.mult)

        # layer-1 per-edge-slot u = x_src * dinv_src (no gather needed)
        dinv_es = scr.tile([128, NCH], BF16, tag="dinv_es")
        rsqrt(dinv_es, deges_t, [128, NCH], "r4", dt=BF16)
        ues = resid.tile([128, NCH, 2], BF16, tag="ues")
        nc.vector.tensor_tensor(
            ues[:], xes_t[:], dinv_es[:, :, None].broadcast_to((128, NCH, 2)), OP.mult)

        W1b = const.tile([2, H], BF16, tag="W1b")
        nc.vector.tensor_copy(W1b[:], W1_t[:])
        W2b = const.tile([H, H], BF16, tag="W2b")
        nc.vector.tensor_copy(W2b[:], W2_t[:])
        W3b = const.tile([H, H], BF16, tag="W3b")
        nc.vector.tensor_copy(W3b[:], W3_t[:])

        sbuild = resid.tile([128, NBLK, H], BF16, tag="sbuild")
        meanp = poolps.tile([128, GPC], F32, tag="meanp")
        pmax = resid.tile([128, NBLK], F32, tag="pmax")

        def build_oh(a0, a1, b0c, b1c):
            """One-hot tile for a group: A chunks then B chunks."""
            na, nb = a1 - a0, b1c - b0c
            oh = ohp.tile([128, GMAXA + GMAXB, 128], BF16, tag="oh")
            if na:
                cb = colidx_t[:, None, :].broadcast_to((128, na, 128))
                db = dlA_t[:, a0:a1, None].broadcast_to((128, na, 128))
                nc.vector.tensor_tensor(oh[:, :na, :], cb, db, OP.is_equal)
            if nb:
                cb = colidx_t[:, None, :].broadcast_to((128, nb, 128))
                db = dlB_t[:, b0c:b1c, None].broadcast_to((128, nb, 128))
                nc.vector.tensor_tensor(oh[:, na:na + nb, :], cb, db, OP.is_equal)
            return oh

        def gather_group(tab, width, a0, a1, b0c, b1c, gmaxa, gmaxb):
            na, nb = a1 - a0, b1c - b0c
            gA = gB = None
            if na:
                gA = gap.tile([128, gmaxa, width], BF16, tag="gA")
                nA = na * 128
                nc.gpsimd.dma_gather(
                    gA[:, :na, :], tab[0:HALF, :],
                    idxA_t[:, a0 * 8: a1 * 8], nA, nA, width,
                    single_packet=SINGLE_PACKET,
                )
            if nb:
                gB = gbp.tile([128, gmaxb, width], BF16, tag="gB")
                nB = nb * 128
                nc.gpsimd.dma_gather(
                    gB[:, :nb, :], tab[HALF:, :],
                    idxB_t[:, b0c * 8: b1c * 8], nB, nB, width,
                    single_packet=SINGLE_PACKET,
                )
            return gA, gB

        # ================= Layer 1 (transposed, rank-2) + prepare L2 ========
        for (b0, b1, a0, a1, bb0, bb1) in groups:
            oh = build_oh(a0, a1, bb0, bb1)
            for b in range(b0, b1):
                # aggUT [2, d] = u_own_blk^T + sum_chunks ues^T onehot-summed
                aggUT_full = aggps.tile([128, 128], F32, tag="agg")
                aggUT = aggUT_full[0:2, :]
                mms = [("self", None)]
                mms += [("A", c) for c in range(int(offA[b]) - a0, int(offA[b + 1]) - a0)]
                mms += [("B", c) for c in range(int(offB[b]) - bb0, int(offB[b + 1]) - bb0)]
                nA = a1 - a0
                for i, (kind, c) in enumerate(mms):
                    st, sp = (i == 0), (i == len(mms) - 1)
                    if kind == "self":
                        nc.tensor.matmul(aggUT, u_own[:, b * 2:(b + 1) * 2],
                                         identb_t[:], start=st, stop=sp)
                    elif kind == "A":
                        nc.tensor.matmul(aggUT, ues[:, a0 + c, :], oh[:, c, :],
                                         start=st, stop=sp)
                    else:
                        nc.tensor.matmul(aggUT, ues[:, NCHA + bb0 + c, :],
                                         oh[:, nA + c, :], start=st, stop=sp)
                cU = wk.tile([2, 128], BF16, tag="cU")
                nc.scalar.copy(cU[:], aggUT)
                hpreT = tps.tile([H, 128], F32, tag="tp")
                nc.tensor.matmul(hpreT[:], W1b[:], cU[:], start=True, stop=True)
                e1 = wk.tile([128, 128], BF16, tag="e1")
                nc.vector.tensor_tensor(
                    e1[:], hpreT[:], dinvrep[:, b * 128:(b + 1) * 128], OP.mult)
                hT = wk.tile([128, 128], BF16, tag="hT")
                nc.scalar.activation(hT[:], e1[:], AF.Tanh, bias=bvec_t[:, 0:1])
                # prepare L2: t1 = (h @ W2) * dinv -> sbuild
                pp = prepps.tile([128, H], F32, tag="pp")
                nc.tensor.matmul(pp[:], hT[:], W2b[:], start=True, stop=True)
                nc.vector.tensor_scalar(
                    sbuild[:, b, :], pp[:], dinv_pm[:, b:b + 1], None, OP.mult)

        nc.sync.dma_start(
            shard_int[0].rearrange("(p b) h -> p (b h)", b=NBLK)[:, :],
            sbuild[:].rearrange("p b h -> p (b h)"),
        )
        nc.gpsimd.collective_compute(
            "AllGather", OP.bypass, replica_groups=[list(range(NCORE))],
            ins=[shard_int[0][:]], outs=[table_int[0][:]],
        )

        # ================= Layer 2 (transposed) + prepare L3 ================
        for (b0, b1, a0, a1, bb0, bb1) in groups:
            gA, gB = gather_group(table_int[0], H, a0, a1, bb0, bb1, GMAXA, GMAXB)
            oh = build_oh(a0, a1, bb0, bb1)
            for b in range(b0, b1):
                aggT = aggps.tile([128, 128], F32, tag="agg")
                mms = [("self", None)]
                mms += [("A", c) for c in range(int(offA[b]) - a0, int(offA[b + 1]) - a0)]
                mms += [("B", c) for c in range(int(offB[b]) - bb0, int(offB[b + 1]) - bb0)]
                nA = a1 - a0
                for i, (kind, c) in enumerate(mms):
                    st, sp = (i == 0), (i == len(mms) - 1)
                    if kind == "self":
                        nc.tensor.matmul(aggT[:], sbuild[:, b, :], identb_t[:],
                                         start=st, stop=sp)
                    elif kind == "A":
                        nc.tensor.matmul(aggT[:], gA[:, c, :], oh[:, c, :],
                                         start=st, stop=sp)
                    else:
                        nc.tensor.matmul(aggT[:], gB[:, c, :], oh[:, nA + c, :],
                                         start=st, stop=sp)
                e1 = wk.tile([128, 128], BF16, tag="e1")
                nc.vector.tensor_tensor(
                    e1[:], aggT[:], dinvrep[:, b * 128:(b + 1) * 128], OP.mult)
                hT = wk.tile([128, 128], BF16, tag="hT")
                nc.scalar.activation(hT[:], e1[:], AF.Tanh, bias=bvec_t[:, 1:2])
                pp = prepps.tile([128, H], F32, tag="pp")
                nc.tensor.matmul(pp[:], hT[:], W3b[:], start=True, stop=True)
                nc.vector.tensor_scalar(
                    sbuild[:, b, :], pp[:], dinv_pm[:, b:b + 1], None, OP.mult)

        nc.sync.dma_start(
            shard_int[1].rearrange("(p b) h -> p (b h)", b=NBLK)[:, :],
            sbuild[:].rearrange("p b h -> p (b h)"),
        )
        nc.gpsimd.collective_compute(
            "AllGather", OP.bypass, replica_groups=[list(range(NCORE))],
            ins=[shard_int[1][:]], outs=[table_int[1][:]],
        )

        # ================= Layer 3 (node-major) + pooling ===================
        for (b0, b1, a0, a1, bb0, bb1) in groups:
            gA, gB = gather_group(table_int[1], H, a0, a1, bb0, bb1, GMAXA, GMAXB)
            oh = build_oh(a0, a1, bb0, bb1)
            for b in range(b0, b1):
                agg = aggps.tile([128, H], F32, tag="agg")
                mms = [("self", None)]
                mms += [("A", c) for c in range(int(offA[b]) - a0, int(offA[b + 1]) - a0)]
                mms += [("B", c) for c in range(int(offB[b]) - bb0, int(offB[b + 1]) - bb0)]
                nA = a1 - a0
                for i, (kind, c) in enumerate(mms):
                    st, sp = (i == 0), (i == len(mms) - 1)
                    if kind == "self":
                        nc.tensor.matmul(agg[:], identb_t[:], sbuild[:, b, :],
                                         start=st, stop=sp)
                    elif kind == "A":
                        nc.tensor.matmul(agg[:], oh[:, c, :], gA[:, c, :],
                                         start=st, stop=sp)
                    else:
                        nc.tensor.matmul(agg[:], oh[:, nA + c, :], gB[:, c, :],
                                         start=st, stop=sp)
                e2 = wk.tile([128, H], F32, tag="e2")
                nc.vector.scalar_tensor_tensor(
                    e2[:], agg[:], dinv_pm[:, b:b + 1], brep3_t[:], OP.mult, OP.add)
                hblk = wk.tile([128, H], F32, tag="hblk")
                nc.scalar.activation(hblk[:], e2[:], AF.Tanh)
                # mean pool accumulate; max pool via PE transpose + free reduce
                nc.tensor.matmul(meanp[:], hblk[:], goh_t[:, b, :],
                                 start=(b == 0), stop=(b == NBLK - 1))
                tp = tps.tile([128, H], F32, tag="tp")
                nc.tensor.transpose(tp[:], hblk[:], ident_t[:])
                nc.vector.tensor_reduce(
                    pmax[:, b:b + 1], tp[:], mybir.AxisListType.X, OP.max)

        # ---- pooling tail + head ----
        p2 = resid.tile([128, NBLK], F32, tag="p2")
        nc.vector.tensor_scalar(p2[:], pmax[:], 2.0, None, OP.add)
        mg = wk.tile([128, GPC, NBLK], F32, tag="mg")
        nc.vector.tensor_tensor(
            mg[:], p2[:, None, :].broadcast_to((128, GPC, NBLK)),
            gmask_t[:].rearrange("p (g b) -> p g b", g=GPC), OP.mult)
        mxT = resid.tile([128, GPC], F32, tag="mxT")
        nc.vector.tensor_reduce(
            mxT[:, :, None], mg[:], mybir.AxisListType.X, OP.max)
        mxT2 = resid.tile([128, GPC], F32, tag="mxT2")
        nc.vector.scalar_tensor_tensor(
            mxT2[:], mxT[:], -2.0, gvalid_t[:], OP.add, OP.mult)

        cmax = wk.tile([128, GPC], F32, tag="cmax")
        nc.vector.tensor_scalar(cmax[:], cntrep_t[:], 1.0, None, OP.max)
        crec = wk.tile([128, GPC], F32, tag="crec")
        nc.vector.reciprocal(crec[:], cmax[:])
        meanT = wk.tile([128, GPC], F32, tag="meanT")
        nc.vector.tensor_tensor(meanT[:], meanp[:], crec[:], OP.mult)

        headp_full = prepps.tile([128, H], F32, tag="pp")
        headp = headp_full[0:GPC, 0:OC]
        nc.tensor.matmul(headp, mxT2[:], Wo_t[:, 0, :], start=True, stop=False)
        nc.tensor.matmul(headp, meanT[:], Wo_t[:, 1, :], start=False, stop=True)
        hsum = wk.tile([GPC, OC], F32, tag="hsum")
        nc.vector.tensor_tensor(hsum[:], headp, bo_t[:], OP.add)
        ofin = wk.tile([GPC, OC], F32, tag="ofin")
        nc.scalar.activation(ofin[:], hsum[:], AF.Tanh)
        nc.sync.dma_start(out_d[:], ofin[:])

    nc.compile()
    return nc


def make_in_maps(meta, inputs):
    colidx = np.tile(np.arange(128, dtype=np.float32), (128, 1)).astype(ml_dtypes.bfloat16)
    identb = np.eye(128, dtype=np.float32).astype(ml_dtypes.bfloat16)
    bvec = np.stack([np.asarray(inputs["b1"], np.float32),
                     np.asarray(inputs["b2"], np.float32)], axis=1)  # [128, 2]
    brep3 = np.tile(np.asarray(inputs["b3"], np.float32), (P, 1))
    bo_t = np.tile(np.asarray(inputs["bo"], np.float32), (GPC, 1))
    Wo = np.asarray(inputs["Wo"], np.float32)
    maps = []
    for c in meta["cores"]:
        maps.append({
            "idxA": c["idxA"], "idxB": c["idxB"],
            "dlA": c["dlA"], "dlB": c["dlB"],
            "colidx": colidx, "identb": identb, "ident": np.eye(128, dtype=np.float32),
            "W1": np.asarray(inputs["W1"], np.float32),
            "W2": np.asarray(inputs["W2"], np.float32),
            "W3": np.asarray(inputs["W3"], np.float32),
            "Wo": np.ascontiguousarray(np.stack([Wo[:H], Wo[H:]], axis=1)),
            "bo": bo_t, "bvec": bvec, "brep3": brep3,
            "gonehot": c["gonehot"], "gmask": c["gmask"], "gvalid": c["gvalid"],
            "cntrep": c["cntrep"],
            "deg_pm": c["deg_pm"], "degrow_rep": c["degrow_rep"],
            "deg_pm2_own": c["deg_pm2_own"], "xnm2_own": c["xnm2_own"],
            "xes": c["xes"], "deges": c["deges"],
        })
    return maps


_CACHE = {}


def kernel(x, edge_index, batch, W1, b1, W2, b2, W3, b3, Wo, bo):
    x = np.asarray(x, np.float32)
    edge_index = np.asarray(edge_index)
    batch = np.asarray(batch)
    meta = prep(x, edge_index, batch, 64)
    key = (meta["NBLK"], meta["NCHA"], meta["NCHB"])
    if key not in _CACHE:
        _CACHE[key] = build(meta)
    nc = _CACHE[key]
    inputs = dict(W1=W1, b1=b1, W2=W2, b2=b2, W3=W3, b3=b3, Wo=Wo, bo=bo)
    in_maps = make_in_maps(meta, inputs)
    res = run_bass_kernel_spmd(nc, in_maps, core_ids=list(range(8)), trace=False)
    out = np.concatenate([res.results[k]["out"] for k in range(8)], 0)
    return np.ascontiguousarray(out, dtype=np.float32)


# revision 29
# speedup vs baseline: 1.0595x; 1.0206x over previous
"""Self-contained Trainium2 Bass kernel for nn_GCNMagnetModel (3-layer GCN,
N=50000 nodes, E=600000 edges, H=128, 64 graphs, 8 NeuronCores, SPMD 1 NEFF).

v3: ZERO collectives. In this environment any collective_compute pins the
per-execution cost at ~3.5ms regardless of size (measured: one 512-byte
AllGather alone = 3.5ms; the whole rest of the kernel < 0.4ms). So layers 1
and 2 are computed REPLICATED over the full graph on every core (layer 1 is
rank-2: agg((x@W1)dinv) == agg(x dinv)@W1, so its per-edge operand is 2-wide
and ships as a host-laid-out input; layer 2's full table is then locally
buildable), and layer 3 + pooling run per-core on the 8 own graphs. No
inter-core communication at all.

- GCN self-loops are ordinary edges in the lists (table row of dst itself),
  so agg = plain one-hot-matmul segment sum; deg = host bincount + 1 (host
  does integer index work only; rsqrt and all FP-on-values is on device).
- Layers 1/2 aggregate TRANSPOSED (stationary = per-edge operand, moving =
  one-hot) so h feeds the next layer's prepare matmul without PE transposes;
  prepare is fused into the same block iteration and writes the next table
  straight to DRAM per block. Layer 3 aggregates node-major for pooling.
- Per-group streaming of idx/dstloc/x_src/deg_src keeps SBUF small.

kernel(**inputs) -> [64, 41] float32.
"""
import numpy as np
import ml_dtypes
from contextlib import ExitStack

import concourse.tile as tile
import concourse.mybir as mybir
from concourse import bacc
from concourse import library_config
from concourse.bass_utils import run_bass_kernel_spmd

NCORE = 8
P = 128
GPC = 8
H = 128
OC = 41

F32 = mybir.dt.float32
BF16 = mybir.dt.bfloat16
I16 = mybir.dt.int16
AF = mybir.ActivationFunctionType
OP = mybir.AluOpType


def wrap16(v):  # [n] -> [128, n/16]: idx[i%16, i//16] tiled 8x
    a = v.reshape(-1, 16).T
    return np.tile(a, (8, 1)).copy()


def prep(x, edge_index, batch, n_graphs=64):
    N = x.shape[0]
    x = np.asarray(x, np.float32)
    batch = np.asarray(batch)
    src_g, dst_g = np.asarray(edge_index[0]), np.asarray(edge_index[1])

    gstart = np.searchsorted(batch, np.arange(n_graphs), side="left")
    gend = np.searchsorted(batch, np.arange(n_graphs), side="right")
    gsz = gend - gstart

    gblk = np.maximum((gsz + P - 1) // P, 1)
    nblk_core = [int(gblk[k * GPC:(k + 1) * GPC].sum()) for k in range(NCORE)]
    NBLK = max(nblk_core)
    NMAXP = NBLK * P
    NBLKG = NCORE * NBLK

    loc_base = np.zeros(n_graphs, np.int64)
    for g in range(n_graphs):
        if g % GPC == 0:
            loc_base[g] = 0
        else:
            loc_base[g] = loc_base[g - 1] + gblk[g - 1] * P
    node_core = batch // GPC
    node_loc = loc_base[batch] + (np.arange(N) - gstart[batch])
    node_row = node_core * NMAXP + node_loc          # node-order global row
    HALF = 4 * NMAXP
    assert HALF < 32768

    deg = np.bincount(dst_g, minlength=N).astype(np.float32) + 1.0

    # append self loops as ordinary edges
    srcs = np.r_[src_g, np.arange(N)]
    dsts = np.r_[dst_g, np.arange(N)]
    r_src = node_row[srcs]
    r_dst = node_row[dsts]
    x_src = x[srcs]
    deg_src = deg[srcs]

    # ---- global (replicated) layout for layers 1+2 ----
    g_bg = r_dst // P
    g_dl = (r_dst % P).astype(np.float32)
    g_half = (r_src >= HALF).astype(np.int64)
    cntsG = np.zeros((NBLKG, 2), np.int64)
    np.add.at(cntsG, (g_bg, g_half), 1)
    cpG = (cntsG + P - 1) // P
    cpG[:, 0] = np.maximum(cpG[:, 0], 1)          # >=1 chunk so agg PSUM is written
    cpGA, cpGB = cpG[:, 0], cpG[:, 1]
    offGA = np.r_[0, np.cumsum(cpGA)]
    offGB = np.r_[0, np.cumsum(cpGB)]
    NCHGA, NCHGB = int(offGA[-1]), int(offGB[-1])
    NCHG = NCHGA + NCHGB

    orderG = np.lexsort((g_bg, g_half))
    so_bg, so_half = g_bg[orderG], g_half[orderG]
    keyG = so_half * NBLKG + so_bg
    rsG = np.r_[0, np.flatnonzero(np.diff(keyG)) + 1]
    ridG = np.zeros(len(orderG), np.int64)
    ridG[rsG[1:]] = 1
    ridG = np.cumsum(ridG)
    posG = np.arange(len(orderG)) - rsG[ridG]
    so_rsrc, so_dl = r_src[orderG], g_dl[orderG]
    so_x, so_degs = x_src[orderG], deg_src[orderG]
    isAG = so_half == 0
    slotG = np.where(isAG, offGA[so_bg] * P + posG, offGB[so_bg] * P + posG)
    gslot = np.where(isAG, slotG, NCHGA * P + slotG)
    idxG = np.zeros(NCHG * P, np.int16)
    dlG = np.full(NCHG * P, -1.0, np.float32)
    xesG = np.zeros((NCHG * P, 2), np.float32)
    degesG = np.ones(NCHG * P, np.float32)
    idxG[gslot] = np.where(isAG, so_rsrc, so_rsrc - HALF).astype(np.int16)
    dlG[gslot] = so_dl
    xesG[gslot] = so_x
    degesG[gslot] = so_degs

    # ---- per-core layout for layer 3 ----
    e_core = r_dst // NMAXP
    l_blk = (r_dst % NMAXP) // P
    cnt3 = np.zeros((NCORE, NBLK, 2), np.int64)
    np.add.at(cnt3, (e_core, l_blk, g_half), 1)
    cp3 = (cnt3.max(axis=0) + P - 1) // P
    cp3[:, 0] = np.maximum(cp3[:, 0], 1)
    cp3A, cp3B = cp3[:, 0], cp3[:, 1]
    off3A = np.r_[0, np.cumsum(cp3A)]
    off3B = np.r_[0, np.cumsum(cp3B)]
    NCH3A, NCH3B = int(off3A[-1]), int(off3B[-1])
    order3 = np.lexsort((l_blk, g_half, e_core))
    s_core, s_blk, s_half = e_core[order3], l_blk[order3], g_half[order3]
    s_rsrc = r_src[order3]
    s_dl = (r_dst % P)[order3].astype(np.float32)
    key3 = (s_core * 2 + s_half) * NBLK + s_blk
    rs3 = np.r_[0, np.flatnonzero(np.diff(key3)) + 1]
    rid3 = np.zeros(len(order3), np.int64)
    rid3[rs3[1:]] = 1
    rid3 = np.cumsum(rid3)
    pos3 = np.arange(len(order3)) - rs3[rid3]
    idx3A = np.zeros((NCORE, NCH3A * P), np.int16)
    idx3B = np.zeros((NCORE, NCH3B * P), np.int16)
    dl3A = np.full((NCORE, NCH3A * P), -1.0, np.float32)
    dl3B = np.full((NCORE, NCH3B * P), -1.0, np.float32)
    isA3 = s_half == 0
    sl3A = off3A[s_blk[isA3]] * P + pos3[isA3]
    idx3A[s_core[isA3], sl3A] = s_rsrc[isA3].astype(np.int16)
    dl3A[s_core[isA3], sl3A] = s_dl[isA3]
    isB3 = ~isA3
    sl3B = off3B[s_blk[isB3]] * P + pos3[isB3]
    idx3B[s_core[isB3], sl3B] = (s_rsrc[isB3] - HALF).astype(np.int16)
    dl3B[s_core[isB3], sl3B] = s_dl[isB3]

    # ---- degree layouts ----
    degrow = np.ones((NCORE, NMAXP), np.float32)
    degrow[node_core, node_loc] = deg
    degrow_flat = degrow.reshape(-1)                        # [NBLKG*128]
    deg_pm_all = np.ascontiguousarray(
        degrow_flat.reshape(NBLKG, P).T)                    # [128, NBLKG] f32
    degrow_rep = np.tile(degrow_flat[None, :], (P, 1)).astype(ml_dtypes.bfloat16)
    deg_pm = [np.ascontiguousarray(degrow[k].reshape(NBLK, P).T) for k in range(NCORE)]

    # ---- pooling masks (per core) ----
    gonehot = np.zeros((NCORE, NBLK * P, GPC), np.float32)
    gmask = np.zeros((NCORE, GPC, NBLK), np.float32)
    for g in range(n_graphs):
        k, gl = g // GPC, g % GPC
        b0 = loc_base[g] // P
        gmask[k, gl, b0:b0 + gblk[g]] = 1.0
        gonehot[k, loc_base[g]:loc_base[g] + gsz[g], gl] = 1.0

    cores = []
    for k in range(NCORE):
        cores.append(dict(
            idx3A=wrap16(idx3A[k]),
            idx3B=wrap16(idx3B[k]),
            dl3A=np.ascontiguousarray(dl3A[k].reshape(NCH3A, P).T).astype(ml_dtypes.bfloat16),
            dl3B=np.ascontiguousarray(dl3B[k].reshape(NCH3B, P).T).astype(ml_dtypes.bfloat16),
            deg_pm=deg_pm[k],
            gonehot=np.ascontiguousarray(
                gonehot[k].reshape(NBLK, P, GPC).transpose(1, 0, 2)).astype(np.float32),
            gmask=np.tile(gmask[k].reshape(1, GPC * NBLK), (P, 1)).astype(np.float32),
            gvalid=np.tile((gsz[k * GPC:(k + 1) * GPC] > 0).astype(np.float32), (P, 1)),
            cntrep=np.tile(gsz[k * GPC:(k + 1) * GPC].astype(np.float32), (P, 1)),
        ))

    meta = dict(
        NBLK=NBLK, NMAXP=NMAXP, HALF=HALF, NBLKG=NBLKG,
        NCHGA=NCHGA, NCHGB=NCHGB,
        offGA=offGA.astype(int), offGB=offGB.astype(int),
        NCH3A=NCH3A, NCH3B=NCH3B,
        off3A=off3A.astype(int), off3B=off3B.astype(int),
        gsz=gsz, cores=cores,
        idxG=wrap16(idxG),                                   # [128, NCHG*8]
        dlG=np.ascontiguousarray(dlG.reshape(NCHG, P).T).astype(ml_dtypes.bfloat16),
        xesG=np.ascontiguousarray(
            xesG.reshape(NCHG, P, 2).transpose(1, 0, 2)).astype(ml_dtypes.bfloat16),
        degesG=np.ascontiguousarray(degesG.reshape(NCHG, P).T).astype(ml_dtypes.bfloat16),
        deg_pm_all=deg_pm_all,
        degrow_rep=degrow_rep,
    )
    return meta


def build(meta, GBLK=8, GBLK3=8, SINGLE_PACKET=False):
    NBLK, NMAXP, HALF, NBLKG = meta["NBLK"], meta["NMAXP"], meta["HALF"], meta["NBLKG"]
    NCHGA, NCHGB = meta["NCHGA"], meta["NCHGB"]
    offGA, offGB = meta["offGA"], meta["offGB"]
    NCH3A, NCH3B = meta["NCH3A"], meta["NCH3B"]
    off3A, off3B = meta["off3A"], meta["off3B"]
    NCHG = NCHGA + NCHGB
    NTAB = NCORE * NMAXP

    groupsG = []
    for g in range((NBLKG + GBLK - 1) // GBLK):
        b0, b1 = g * GBLK, min((g + 1) * GBLK, NBLKG)
        groupsG.append((b0, b1, int(offGA[b0]), int(offGA[b1]),
                        int(offGB[b0]), int(offGB[b1])))
    GMAXA = max(a1 - a0 for (_, _, a0, a1, _, _) in groupsG)
    GMAXB = max(x1 - x0 for (_, _, _, _, x0, x1) in groupsG)
    groups3 = []
    for g in range((NBLK + GBLK3 - 1) // GBLK3):
        b0, b1 = g * GBLK3, min((g + 1) * GBLK3, NBLK)
        groups3.append((b0, b1, int(off3A[b0]), int(off3A[b1]),
                        int(off3B[b0]), int(off3B[b1])))
    G3MAXA = max(a1 - a0 for (_, _, a0, a1, _, _) in groups3)
    G3MAXB = max(x1 - x0 for (_, _, _, _, x0, x1) in groups3)

    nc = bacc.Bacc(None, target_bir_lowering=False)

    # ---- IO ----
    idxG_d = nc.dram_tensor("idxG", [128, NCHG * 8], I16, kind="ExternalInput")
    dlG_d = nc.dram_tensor("dlG", [128, NCHG], BF16, kind="ExternalInput")
    xesG_d = nc.dram_tensor("xesG", [128, NCHG, 2], BF16, kind="ExternalInput")
    degesG_d = nc.dram_tensor("degesG", [128, NCHG], BF16, kind="ExternalInput")
    deg_pm_all_d = nc.dram_tensor("deg_pm_all", [128, NBLKG], F32, kind="ExternalInput")
    degrow_rep_d = nc.dram_tensor("degrow_rep", [128, NTAB], BF16, kind="ExternalInput")
    idx3A_d = nc.dram_tensor("idx3A", [128, NCH3A * 8], I16, kind="ExternalInput")
    idx3B_d = nc.dram_tensor("idx3B", [128, NCH3B * 8], I16, kind="ExternalInput")
    dl3A_d = nc.dram_tensor("dl3A", [128, NCH3A], BF16, kind="ExternalInput")
    dl3B_d = nc.dram_tensor("dl3B", [128, NCH3B], BF16, kind="ExternalInput")
    deg_pm_d = nc.dram_tensor("deg_pm", [128, NBLK], F32, kind="ExternalInput")
    colidx_d = nc.dram_tensor("colidx", [128, 128], BF16, kind="ExternalInput")
    ident_d = nc.dram_tensor("ident", [128, 128], F32, kind="ExternalInput")
    W1_d = nc.dram_tensor("W1", [2, H], F32, kind="ExternalInput")
    W2_d = nc.dram_tensor("W2", [H, H], F32, kind="ExternalInput")
    W3_d = nc.dram_tensor("W3", [H, H], F32, kind="ExternalInput")
    Wo_d = nc.dram_tensor("Wo", [H, 2, OC], F32, kind="ExternalInput")
    bo_d = nc.dram_tensor("bo", [GPC, OC], F32, kind="ExternalInput")
    bvec_d = nc.dram_tensor("bvec", [128, 2], F32, kind="ExternalInput")
    brep3_d = nc.dram_tensor("brep3", [128, H], F32, kind="ExternalInput")
    goh_d = nc.dram_tensor("gonehot", [128, NBLK, GPC], F32, kind="ExternalInput")
    gmask_d = nc.dram_tensor("gmask", [128, GPC * NBLK], F32, kind="ExternalInput")
    gvalid_d = nc.dram_tensor("gvalid", [128, GPC], F32, kind="ExternalInput")
    cntrep_d = nc.dram_tensor("cntrep", [128, GPC], F32, kind="ExternalInput")
    out_d = nc.dram_tensor("out", [GPC, OC], F32, kind="ExternalOutput")

    table2_d = nc.dram_tensor("table2", [NTAB, H], BF16)
    table3_d = nc.dram_tensor("table3", [NTAB, H], BF16)

    with tile.TileContext(nc) as tc, ExitStack() as ctx:
        const = ctx.enter_context(tc.tile_pool(name="const", bufs=1))
        resid = ctx.enter_context(tc.tile_pool(name="resid", bufs=1))
        strm = ctx.enter_context(tc.tile_pool(name="strm", bufs=2))
        gap = ctx.enter_context(tc.tile_pool(name="gap", bufs=2))
        gbp = ctx.enter_context(tc.tile_pool(name="gbp", bufs=2))
        ohp = ctx.enter_context(tc.tile_pool(name="ohp", bufs=2))
        wk = ctx.enter_context(tc.tile_pool(name="wk", bufs=3))
        aggps = ctx.enter_context(tc.tile_pool(name="aggps", bufs=3, space="PSUM"))
        prepps = ctx.enter_context(tc.tile_pool(name="prepps", bufs=2, space="PSUM"))
        tps = ctx.enter_context(tc.tile_pool(name="tps", bufs=2, space="PSUM"))
        poolps = ctx.enter_context(tc.tile_pool(name="poolps", bufs=1, space="PSUM"))

        nc.gpsimd.load_library(library_config.mlp)

        def load_const(dram, shape, dt):
            t = const.tile(shape, dt, tag=dram.name)
            nc.sync.dma_start(t[:], dram[:])
            return t

        idx3A_t = load_const(idx3A_d, [128, NCH3A * 8], I16)
        idx3B_t = load_const(idx3B_d, [128, NCH3B * 8], I16)
        dl3A_t = load_const(dl3A_d, [128, NCH3A], BF16)
        dl3B_t = load_const(dl3B_d, [128, NCH3B], BF16)
        deg_pm_all_t = load_const(deg_pm_all_d, [128, NBLKG], F32)
        deg_pm_t = load_const(deg_pm_d, [128, NBLK], F32)
        colidx_t = load_const(colidx_d, [128, 128], BF16)
        ident_t = load_const(ident_d, [128, 128], F32)
        W1_t = load_const(W1_d, [2, H], F32)
        W2_t = load_const(W2_d, [H, H], F32)
        W3_t = load_const(W3_d, [H, H], F32)
        Wo_t = load_const(Wo_d, [H, 2, OC], F32)
        bo_t = load_const(bo_d, [GPC, OC], F32)
        bvec_t = load_const(bvec_d, [128, 2], F32)
        brep3_t = load_const(brep3_d, [128, H], F32)
        goh_t = load_const(goh_d, [128, NBLK, GPC], F32)
        gmask_t = load_const(gmask_d, [128, GPC * NBLK], F32)
        gvalid_t = load_const(gvalid_d, [128, GPC], F32)
        cntrep_t = load_const(cntrep_d, [128, GPC], F32)

        # dinv for prepare scaling (partition-major over all global blocks)
        dinv_all = resid.tile([128, NBLKG], F32, tag="dinv_all")
        rec_all = resid.tile([128, NBLKG], F32, tag="rec_all")
        nc.vector.reciprocal(rec_all[:], deg_pm_all_t[:])
        nc.scalar.activation(dinv_all[:], rec_all[:], AF.Sqrt)
        dinv_pm = resid.tile([128, NBLK], F32, tag="dinv_pm")
        rec_pm = resid.tile([128, NBLK], F32, tag="rec_pm")
        nc.vector.reciprocal(rec_pm[:], deg_pm_t[:])
        nc.scalar.activation(dinv_pm[:], rec_pm[:], AF.Sqrt)

        W1b = const.tile([2, H], BF16, tag="W1b")
        nc.vector.tensor_copy(W1b[:], W1_t[:])
        W2b = const.tile([H, H], BF16, tag="W2b")
        nc.vector.tensor_copy(W2b[:], W2_t[:])
        W3b = const.tile([H, H], BF16, tag="W3b")
        nc.vector.tensor_copy(W3b[:], W3_t[:])

        meanp = poolps.tile([128, GPC], F32, tag="meanp")
        pmax = resid.tile([128, NBLK], F32, tag="pmax")

        def build_oh(dlA_ap, dlB_ap, na, nb, gma, gmb):
            oh = ohp.tile([128, gma + gmb, 128], BF16, tag="oh")
            if na:
                cb = colidx_t[:, None, :].broadcast_to((128, na, 128))
                db = dlA_ap[:, :, None].broadcast_to((128, na, 128))
                nc.vector.tensor_tensor(oh[:, :na, :], cb, db, OP.is_equal)
            if nb:
                cb = colidx_t[:, None, :].broadcast_to((128, nb, 128))
                db = dlB_ap[:, :, None].broadcast_to((128, nb, 128))
                nc.vector.tensor_tensor(oh[:, na:na + nb, :], cb, db, OP.is_equal)
            return oh

        def dinvrep_group(b0, b1):
            n = (b1 - b0) * 128
            dg = strm.tile([128, GBLK * 128], BF16, tag="dg")
            nc.sync.dma_start(dg[:, :n], degrow_rep_d[:, b0 * 128:b1 * 128])
            rg = strm.tile([128, GBLK * 128], BF16, tag="rg")
            with nc.allow_low_precision(reason="deg exact in bf16"):
                nc.vector.reciprocal(rg[:, :n], dg[:, :n])
            dr = strm.tile([128, GBLK * 128], BF16, tag="dr")
            nc.scalar.activation(dr[:, :n], rg[:, :n], AF.Sqrt)
            return dr

        # ================= Layer 1 (replicated, rank-2) + table2 ============
        for (b0, b1, a0, a1, x0, x1) in groupsG:
            na, nb = a1 - a0, x1 - x0
            dlAg = strm.tile([128, GMAXA], BF16, tag="dlAg")
            if na:
                nc.sync.dma_start(dlAg[:, :na], dlG_d[:, a0:a1])
            dlBg = strm.tile([128, GMAXB], BF16, tag="dlBg")
            if nb:
                nc.sync.dma_start(dlBg[:, :nb], dlG_d[:, NCHGA + x0:NCHGA + x1])
            xeg = strm.tile([128, GMAXA + GMAXB, 2], BF16, tag="xeg")
            deg_g = strm.tile([128, GMAXA + GMAXB], BF16, tag="deg_g")
            if na:
                nc.sync.dma_start(xeg[:, :na, :], xesG_d[:, a0:a1, :])
                nc.sync.dma_start(deg_g[:, :na], degesG_d[:, a0:a1])
            if nb:
                nc.sync.dma_start(xeg[:, na:na + nb, :], xesG_d[:, NCHGA + x0:NCHGA + x1, :])
                nc.sync.dma_start(deg_g[:, na:na + nb], degesG_d[:, NCHGA + x0:NCHGA + x1])
            nch = na + nb
            rg1 = strm.tile([128, GMAXA + GMAXB], BF16, tag="rg1")
            with nc.allow_low_precision(reason="deg exact in bf16"):
                nc.vector.reciprocal(rg1[:, :nch], deg_g[:, :nch])
            dsg = strm.tile([128, GMAXA + GMAXB], BF16, tag="dsg")
            nc.scalar.activation(dsg[:, :nch], rg1[:, :nch], AF.Sqrt)
            ueg = strm.tile([128, GMAXA + GMAXB, 2], BF16, tag="ueg")
            nc.vector.tensor_tensor(
                ueg[:, :nch, :], xeg[:, :nch, :],
                dsg[:, :nch, None].broadcast_to((128, nch, 2)), OP.mult)

            oh = build_oh(dlAg[:, :na] if na else None,
                          dlBg[:, :nb] if nb else None, na, nb, GMAXA, GMAXB)
            dr = dinvrep_group(b0, b1)
            for b in range(b0, b1):
                aggUT_full = aggps.tile([128, 128], F32, tag="agg")
                aggUT = aggUT_full[0:2, :]
                cks = [c for c in range(int(offGA[b]) - a0, int(offGA[b + 1]) - a0)]
                cks += [na + c for c in range(int(offGB[b]) - x0, int(offGB[b + 1]) - x0)]
                for i, c in enumerate(cks):
                    nc.tensor.matmul(aggUT, ueg[:, c, :], oh[:, c, :],
                                     start=(i == 0), stop=(i == len(cks) - 1))
                cU = wk.tile([2, 128], BF16, tag="cU")
                nc.scalar.copy(cU[:], aggUT)
                hpreT = tps.tile([H, 128], F32, tag="tp")
                nc.tensor.matmul(hpreT[:], W1b[:], cU[:], start=True, stop=True)
                e1 = wk.tile([128, 128], BF16, tag="e1")
                nc.vector.tensor_tensor(
                    e1[:], hpreT[:], dr[:, (b - b0) * 128:(b - b0 + 1) * 128], OP.mult)
                hT = wk.tile([128, 128], BF16, tag="hT")
                nc.scalar.activation(hT[:], e1[:], AF.Tanh, bias=bvec_t[:, 0:1])
                pp = prepps.tile([128, H], F32, tag="pp")
                nc.tensor.matmul(pp[:], hT[:], W2b[:], start=True, stop=True)
                t12 = wk.tile([128, H], BF16, tag="t12")
                nc.vector.tensor_scalar(
                    t12[:], pp[:], dinv_all[:, b:b + 1], None, OP.mult)
                nc.sync.dma_start(table2_d[b * 128:(b + 1) * 128, :], t12[:])

        # ================= Layer 2 (replicated) + table3 ====================
        for (b0, b1, a0, a1, x0, x1) in groupsG:
            na, nb = a1 - a0, x1 - x0
            dlAg = strm.tile([128, GMAXA], BF16, tag="dlAg")
            if na:
                nc.sync.dma_start(dlAg[:, :na], dlG_d[:, a0:a1])
            dlBg = strm.tile([128, GMAXB], BF16, tag="dlBg")
            if nb:
                nc.sync.dma_start(dlBg[:, :nb], dlG_d[:, NCHGA + x0:NCHGA + x1])
            gA = gB = None
            if na:
                ixA = strm.tile([128, GMAXA * 8], I16, tag="ixA")
                nc.sync.dma_start(ixA[:, :na * 8], idxG_d[:, a0 * 8:a1 * 8])
                gA = gap.tile([128, GMAXA, H], BF16, tag="gA")
                nc.gpsimd.dma_gather(
                    gA[:, :na, :], table2_d[0:HALF, :],
                    ixA[:, :na * 8], na * 128, na * 128, H,
                    single_packet=SINGLE_PACKET)
            if nb:
                ixB = strm.tile([128, GMAXB * 8], I16, tag="ixB")
                nc.sync.dma_start(ixB[:, :nb * 8], idxG_d[:, (NCHGA + x0) * 8:(NCHGA + x1) * 8])
                gB = gbp.tile([128, GMAXB, H], BF16, tag="gB")
                nc.gpsimd.dma_gather(
                    gB[:, :nb, :], table2_d[HALF:, :],
                    ixB[:, :nb * 8], nb * 128, nb * 128, H,
                    single_packet=SINGLE_PACKET)
            oh = build_oh(dlAg[:, :na] if na else None,
                          dlBg[:, :nb] if nb else None, na, nb, GMAXA, GMAXB)
            dr = dinvrep_group(b0, b1)
            for b in range(b0, b1):
                aggT = aggps.tile([128, 128], F32, tag="agg")
                mms = [(gA, c, c) for c in range(int(offGA[b]) - a0, int(offGA[b + 1]) - a0)]
                mms += [(gB, c, na + c) for c in range(int(offGB[b]) - x0, int(offGB[b + 1]) - x0)]
                for i, (gt, c, co) in enumerate(mms):
                    nc.tensor.matmul(aggT[:], gt[:, c, :], oh[:, co, :],
                                     start=(i == 0), stop=(i == len(mms) - 1))
                e1 = wk.tile([128, 128], BF16, tag="e1")
                nc.vector.tensor_tensor(
                    e1[:], aggT[:], dr[:, (b - b0) * 128:(b - b0 + 1) * 128], OP.mult)
                hT = wk.tile([128, 128], BF16, tag="hT")
                nc.scalar.activation(hT[:], e1[:], AF.Tanh, bias=bvec_t[:, 1:2])
                pp = prepps.tile([128, H], F32, tag="pp")
                nc.tensor.matmul(pp[:], hT[:], W3b[:], start=True, stop=True)
                t13 = wk.tile([128, H], BF16, tag="t13")
                nc.vector.tensor_scalar(
                    t13[:], pp[:], dinv_all[:, b:b + 1], None, OP.mult)
                nc.sync.dma_start(table3_d[b * 128:(b + 1) * 128, :], t13[:])

        # ================= Layer 3 (per-core, node-major) + pooling =========
        for (b0, b1, a0, a1, x0, x1) in groups3:
            na, nb = a1 - a0, x1 - x0
            gA = gB = None
            if na:
                gA = gap.tile([128, G3MAXA, H], BF16, tag="gA")
                nc.gpsimd.dma_gather(
                    gA[:, :na, :], table3_d[0:HALF, :],
                    idx3A_t[:, a0 * 8:a1 * 8], na * 128, na * 128, H,
                    single_packet=SINGLE_PACKET)
            if nb:
                gB = gbp.tile([128, G3MAXB, H], BF16, tag="gB")
                nc.gpsimd.dma_gather(
                    gB[:, :nb, :], table3_d[HALF:, :],
                    idx3B_t[:, x0 * 8:x1 * 8], nb * 128, nb * 128, H,
                    single_packet=SINGLE_PACKET)
            oh = build_oh(dl3A_t[:, a0:a1] if na else None,
                          dl3B_t[:, x0:x1] if nb else None, na, nb, G3MAXA, G3MAXB)
            for b in range(b0, b1):
                agg = aggps.tile([128, H], F32, tag="agg")
                mms = [(gA, c, c) for c in range(int(off3A[b]) - a0, int(off3A[b + 1]) - a0)]
                mms += [(gB, c, na + c) for c in range(int(off3B[b]) - x0, int(off3B[b + 1]) - x0)]
                for i, (gt, c, co) in enumerate(mms):
                    nc.tensor.matmul(agg[:], oh[:, co, :], gt[:, c, :],
                                     start=(i == 0), stop=(i == len(mms) - 1))
                e2 = wk.tile([128, H], F32, tag="e2")
                nc.vector.scalar_tensor_tensor(
                    e2[:], agg[:], dinv_pm[:, b:b + 1], brep3_t[:], OP.mult, OP.add)
                hblk = wk.tile([128, H], F32, tag="hblk")
                nc.scalar.activation(hblk[:], e2[:], AF.Tanh)
                nc.tensor.matmul(meanp[:], hblk[:], goh_t[:, b, :],
                                 start=(b == 0), stop=(b == NBLK - 1))
                tp = tps.tile([128, H], F32, tag="tp")
                nc.tensor.transpose(tp[:], hblk[:], ident_t[:])
                nc.vector.tensor_reduce(
                    pmax[:, b:b + 1], tp[:], mybir.AxisListType.X, OP.max)

        # ---- pooling tail + head ----
        p2 = resid.tile([128, NBLK], F32, tag="p2")
        nc.vector.tensor_scalar(p2[:], pmax[:], 2.0, None, OP.add)
        mg = wk.tile([128, GPC, NBLK], F32, tag="mg")
        nc.vector.tensor_tensor(
            mg[:], p2[:, None, :].broadcast_to((128, GPC, NBLK)),
            gmask_t[:].rearrange("p (g b) -> p g b", g=GPC), OP.mult)
        mxT = resid.tile([128, GPC], F32, tag="mxT")
        nc.vector.tensor_reduce(
            mxT[:, :, None], mg[:], mybir.AxisListType.X, OP.max)
        mxT2 = resid.tile([128, GPC], F32, tag="mxT2")
        nc.vector.scalar_tensor_tensor(
            mxT2[:], mxT[:], -2.0, gvalid_t[:], OP.add, OP.mult)

        cmax = wk.tile([128, GPC], F32, tag="cmax")
        nc.vector.tensor_scalar(cmax[:], cntrep_t[:], 1.0, None, OP.max)
        crec = wk.tile([128, GPC], F32, tag="crec")
        nc.vector.reciprocal(crec[:], cmax[:])
        meanT = wk.tile([128, GPC], F32, tag="meanT")
        nc.vector.tensor_tensor(meanT[:], meanp[:], crec[:], OP.mult)

        headp_full = prepps.tile([128, H], F32, tag="pp")
        headp = headp_full[0:GPC, 0:OC]
        nc.tensor.matmul(headp, mxT2[:], Wo_t[:, 0, :], start=True, stop=False)
        nc.tensor.matmul(headp, meanT[:], Wo_t[:, 1, :], start=False, stop=True)
        hsum = wk.tile([GPC, OC], F32, tag="hsum")
        nc.vector.tensor_tensor(hsum[:], headp, bo_t[:], OP.add)
        ofin = wk.tile([GPC, OC], F32, tag="ofin")
        nc.scalar.activation(ofin[:], hsum[:], AF.Tanh)
        nc.sync.dma_start(out_d[:], ofin[:])

    nc.compile()
    return nc


def make_in_maps(meta, inputs):
    colidx = np.tile(np.arange(128, dtype=np.float32), (128, 1)).astype(ml_dtypes.bfloat16)
    bvec = np.stack([np.asarray(inputs["b1"], np.float32),
                     np.asarray(inputs["b2"], np.float32)], axis=1)
    brep3 = np.tile(np.asarray(inputs["b3"], np.float32), (P, 1))
    bo_t = np.tile(np.asarray(inputs["bo"], np.float32), (GPC, 1))
    Wo = np.asarray(inputs["Wo"], np.float32)
    shared = {
        "idxG": meta["idxG"], "dlG": meta["dlG"],
        "xesG": meta["xesG"], "degesG": meta["degesG"],
        "deg_pm_all": meta["deg_pm_all"], "degrow_rep": meta["degrow_rep"],
        "colidx": colidx, "ident": np.eye(128, dtype=np.float32),
        "W1": np.asarray(inputs["W1"], np.float32),
        "W2": np.asarray(inputs["W2"], np.float32),
        "W3": np.asarray(inputs["W3"], np.float32),
        "Wo": np.ascontiguousarray(np.stack([Wo[:H], Wo[H:]], axis=1)),
        "bo": bo_t, "bvec": bvec, "brep3": brep3,
    }
    maps = []
    for c in meta["cores"]:
        m = dict(shared)
        m.update({
            "idx3A": c["idx3A"], "idx3B": c["idx3B"],
            "dl3A": c["dl3A"], "dl3B": c["dl3B"],
            "deg_pm": c["deg_pm"],
            "gonehot": c["gonehot"], "gmask": c["gmask"],
            "gvalid": c["gvalid"], "cntrep": c["cntrep"],
        })
        maps.append(m)
    return maps


_CACHE = {}


def kernel(x, edge_index, batch, W1, b1, W2, b2, W3, b3, Wo, bo):
    x = np.asarray(x, np.float32)
    edge_index = np.asarray(edge_index)
    batch = np.asarray(batch)
    meta = prep(x, edge_index, batch, 64)
    key = (meta["NBLK"], meta["NCHGA"], meta["NCHGB"], meta["NCH3A"], meta["NCH3B"])
    if key not in _CACHE:
        _CACHE[key] = build(meta)
    nc = _CACHE[key]
    inputs = dict(W1=W1, b1=b1, W2=W2, b2=b2, W3=W3, b3=b3, Wo=Wo, bo=bo)
    in_maps = make_in_maps(meta, inputs)
    res = run_bass_kernel_spmd(nc, in_maps, core_ids=list(range(8)), trace=False)
    out = np.concatenate([res.results[k]["out"] for k in range(8)], 0)
    return np.ascontiguousarray(out, dtype=np.float32)
